# revision 2
# baseline (speedup 1.0000x reference)
"""Trainium2 Bass kernel for KBLAM Gemma3n attention (B=2, S=1024, H=2048,
NH=16, NKV=4, HD=128, KB=1024), sharded over 8 NeuronCores as
(batch x kv-head-group): core = 4*b + g handles batch b and kv head g
(which serves q-heads 4g..4g+3).  Each core computes a partial s-major
output y_part (S, H) = attn_out @ Wo[:, 512g:512g+512].T ; the host sums
the 4 partials per batch.

Device-side design (matmul operands in float32r: 4x the fp32 PE rate at
~1.6e-4 matmul relative error):
 - projections produce d-major tensors qT/qnT [128, 4096] (head i at cols
   1024i), kT [128,1024], via out = lhsT.T @ rhs with lhsT = W^T tile [h,d],
   rhs = x^T tile [h,s].  Weights are host-packed in per-dt tile order so
   each projection chunk is one contiguous [128, 2048] DMA.
 - RoPE via a permutation matmul (P @ qT) plus 3 DVE ops per 512-chunk.
 - scores are computed transposed: scoresT[key, q] = kT_tile.T @ qT_chunk, so
   softmax-exp output attnT feeds attn@v directly as the moving operand with
   v in natural key-major layout (no transposes in the attention inner loop).
 - softmax denominators via ones-column matmuls accumulated alongside attn@v;
   normalization via fast-approx reciprocal + ones-row broadcast matmul +
   DVE multiply.
 - attention_mask handled generally: each 128-key x 512-query self block is
   classified on host as fully-masked (skipped), zero (no mask add), or mixed
   (additive mask tile DMA'd and added before exp).
"""
import math
from contextlib import ExitStack

import numpy as np

B, S, H = 2, 1024, 2048
NH, NKV, HD = 16, 4, 128
KB = 1024
THETA = 10000.0
SCALE = 1.0 / math.sqrt(HD)


def _build_program(self_tiles, mixed_idx, n_mask, col0_map):
    """Build the single-core Bass/Tile program.

    self_tiles: {c: [t, ...]} self-attention key tiles to compute per q-chunk
    mixed_idx: {(t, c): k} index into the packed mask tensor for mixed blocks
    n_mask: number of packed [128, 512] mask tiles (0 if none)
    """
    import concourse.tile as tile
    from concourse import bacc, mybir

    f32 = mybir.dt.float32
    f32r = mybir.dt.float32r
    nc = bacc.Bacc("TRN2", target_bir_lowering=False, debug=False,
                   enable_asserts=False, num_devices=8)

    xT = nc.dram_tensor("xT", [H, S], f32r, kind="ExternalInput")
    # packed weights: per-dt blocks of 16 h-tiles: cols 2048*dt + 128*h
    wq = nc.dram_tensor("wq", [128, 8192], f32r, kind="ExternalInput")
    wqn = nc.dram_tensor("wqn", [128, 8192], f32r, kind="ExternalInput")
    wk = nc.dram_tensor("wk", [128, 2048], f32r, kind="ExternalInput")
    wv = nc.dram_tensor("wv", [128, 2048], f32r, kind="ExternalInput")
    # wo packed: block i at cols 2048*i = Wo_g^T[128i:128i+128, :]
    wo = nc.dram_tensor("wo", [128, 8192], f32r, kind="ExternalInput")
    kbkT = nc.dram_tensor("kbkT", [128, KB], f32r, kind="ExternalInput")
    # kbv packed key-major tiles side by side: tile t at cols 128*t
    kbv = nc.dram_tensor("kbv", [128, KB], f32r, kind="ExternalInput")
    cosT = nc.dram_tensor("cosT", [128, S], f32r, kind="ExternalInput")
    sinT = nc.dram_tensor("sinT", [128, S], f32r, kind="ExternalInput")
    ropePT = nc.dram_tensor("ropePT", [128, 128], f32r, kind="ExternalInput")
    ones = nc.dram_tensor("ones", [128, 128], f32r, kind="ExternalInput")
    ident = nc.dram_tensor("ident", [128, 128], f32r, kind="ExternalInput")
    if n_mask:
        masks = nc.dram_tensor("masks", [128, 512 * n_mask], f32,
                               kind="ExternalInput")
    y = nc.dram_tensor("y", [S, H], f32, kind="ExternalOutput")

    with tile.TileContext(nc) as tc, ExitStack() as ctx:
        po = ctx.enter_context(tc.tile_pool(name="projout", bufs=1))
        qTr = po.tile([128, 4096], f32r, tag="qTr")
        qnT = po.tile([128, 4096], f32r, tag="qnT")
        kTr = po.tile([128, 1024], f32r, tag="kTr")
        vkm = po.tile([128, 1024], f32r, tag="vkm")

        consts = ctx.enter_context(tc.tile_pool(name="consts", bufs=1))
        kbp = ctx.enter_context(tc.tile_pool(name="kb", bufs=1))

        # ---------------- phase 1: projections + rope + v transpose ------
        with tc.tile_pool(name="xw", bufs=1) as xw, \
             tc.tile_pool(name="wt", bufs=3) as wpool, \
             tc.tile_pool(name="ptmp", bufs=3) as ptmp, \
             tc.tile_pool(name="psp", bufs=2, space="PSUM") as psp, \
             tc.tile_pool(name="psr", bufs=2, space="PSUM") as psr:
            # weight blocks for the first two chunks (k, v) load BEFORE
            # the big xT transfer so the PE can start at ~3us.
            wblk_k = wpool.tile([128, 2048], f32r, tag="wblk", name="wblk_k")
            nc.sync.dma_start(wblk_k[:], wk[:])
            wblk_v = wpool.tile([128, 2048], f32r, tag="wblk", name="wblk_v")
            nc.sync.dma_start(wblk_v[:], wv[:])
            xt = xw.tile([128, 16384], f32r, tag="xt")
            for h in range(16):
                nc.sync.dma_start(xt[:, 1024 * h:1024 * h + 1024],
                                  xT[128 * h:128 * h + 128, :])
            vt_tmp = xw.tile([128, 1024], f32r, tag="vt")

            cos_sb = consts.tile([128, S], f32r, tag="cos")
            nc.sync.dma_start(cos_sb[:], cosT[:])
            sin_sb = consts.tile([128, S], f32r, tag="sin")
            nc.sync.dma_start(sin_sb[:], sinT[:])
            rp_sb = consts.tile([128, 128], f32r, tag="rp")
            nc.sync.dma_start(rp_sb[:], ropePT[:])
            id_sb = consts.tile([128, 128], f32r, tag="id")
            nc.sync.dma_start(id_sb[:], ident[:])

            def rope_chunk(ps, half, dst):
                tmp = ptmp.tile([128, 512], f32r, tag="tmp")
                nc.any.tensor_copy(tmp[:], ps[:])
                pp = psr.tile([128, 512], f32, tag="pp")
                nc.tensor.matmul(pp[:], rp_sb[:], tmp[:], start=True, stop=True)
                cs = cos_sb[:, 512 * half:512 * half + 512]
                sn = sin_sb[:, 512 * half:512 * half + 512]
                nc.vector.tensor_mul(dst, tmp[:], cs)
                tmp2 = ptmp.tile([128, 512], f32r, tag="tmp2")
                nc.vector.tensor_mul(tmp2[:], pp[:], sn)
                nc.vector.tensor_add(dst, dst, tmp2[:])

            # order: k, v first, then (q, qn) per head: lets attention for
            # head i start while head i+1 is still projecting.
            chunks = [(wk, 0, 'k'), (wv, 0, 'v')]
            for i in range(4):
                chunks.append((wq, i, 'q'))
                chunks.append((wqn, i, 'qn'))
            for ci, (w_dram, dt_i, kind) in enumerate(chunks):
                if ci == 0:
                    wblk = wblk_k
                elif ci == 1:
                    wblk = wblk_v
                else:
                    wblk = wpool.tile([128, 2048], f32r, tag="wblk",
                                      name="wblk")
                    nc.sync.dma_start(wblk[:],
                                      w_dram[:, 2048 * dt_i:2048 * dt_i + 2048])
                pss = [psp.tile([128, 512], f32, tag="pp0", name="pp0"),
                       psp.tile([128, 512], f32, tag="pp1", name="pp1")]
                for h in range(16):
                    for half in range(2):
                        nc.tensor.matmul(
                            pss[half][:], wblk[:, 128 * h:128 * h + 128],
                            xt[:, 1024 * h + 512 * half:
                               1024 * h + 512 * half + 512],
                            start=(h == 0), stop=(h == 15))
                for half in range(2):
                    if kind == 'q':
                        dst = qTr[:, 1024 * dt_i + 512 * half:
                                  1024 * dt_i + 512 * half + 512]
                        rope_chunk(pss[half], half, dst)
                    elif kind == 'k':
                        dst = kTr[:, 512 * half:512 * half + 512]
                        rope_chunk(pss[half], half, dst)
                    elif kind == 'qn':
                        nc.any.tensor_copy(
                            qnT[:, 1024 * dt_i + 512 * half:
                                1024 * dt_i + 512 * half + 512],
                            pss[half][:])
                    else:  # v
                        nc.any.tensor_copy(
                            vt_tmp[:, 512 * half:512 * half + 512],
                            pss[half][:])
                if kind == 'v':
                    for t in range(8):
                        pst = psr.tile([128, 128], f32r, tag="ptr")
                        nc.tensor.transpose(
                            pst[:], vt_tmp[:, 128 * t:128 * t + 128], id_sb[:])
                        nc.any.tensor_copy(vkm[:, 128 * t:128 * t + 128],
                                           pst[:])

            # loads needed by the attention phase (emitted late so the
            # projection-critical DMAs win the early queue slots)
            ones_sb = consts.tile([128, 128], f32r, tag="ones")
            nc.sync.dma_start(ones_sb[:], ones[:])
            if n_mask:
                mask_sb = consts.tile([128, 512 * n_mask], f32, tag="mask")
                nc.sync.dma_start(mask_sb[:], masks[:])
            kbk_sb = kbp.tile([128, KB], f32r, tag="kbk")
            nc.sync.dma_start(kbk_sb[:], kbkT[:])
            kbv_sb = kbp.tile([128, KB], f32r, tag="kbv")
            nc.sync.dma_start(kbv_sb[:], kbv[:])

        # ---------------- phase 2: attention ------------------------------
        onp = ctx.enter_context(tc.tile_pool(name="onp", bufs=1))
        outn = onp.tile([128, 4096], f32r, tag="outn")
        wo_sb = onp.tile([128, 8192], f32r, tag="wo")
        nc.sync.dma_start(wo_sb[:], wo[:])

        with tc.tile_pool(name="at", bufs=6) as atp, \
             tc.tile_pool(name="nrm", bufs=3) as nrm, \
             tc.tile_pool(name="pssc", bufs=3, space="PSUM") as pssc, \
             tc.tile_pool(name="psout", bufs=2, space="PSUM") as psout, \
             tc.tile_pool(name="psden", bufs=1, space="PSUM") as psden, \
             tc.tile_pool(name="psbc", bufs=1, space="PSUM") as psbc, \
             tc.tile_pool(name="psy", bufs=1, space="PSUM") as psy, \
             tc.tile_pool(name="ysb", bufs=2) as ysbp:

            def emit_y_tile(st):
                cy, off = st // 4, 128 * (st % 4)
                ysb = ysbp.tile([128, 2048], f32, tag="ysb", name="ysb")
                for n in range(4):
                    py = psy.tile([128, 512], f32, tag="y", name="py")
                    for i in range(4):
                        lcol = 1024 * i + 512 * cy + off
                        nc.tensor.matmul(
                            py[:], outn[:, lcol:lcol + 128],
                            wo_sb[:, 2048 * i + 512 * n:
                                  2048 * i + 512 * n + 512],
                            start=(i == 0), stop=(i == 3))
                    nc.vector.tensor_copy(ysb[:, 512 * n:512 * n + 512],
                                          py[:])
                nc.sync.dma_start(y[128 * st:128 * st + 128, :], ysb[:])

            for c in range(2):
                for i in range(4):
                    qcol = 1024 * i + 512 * c
                    steps = [('kb', t) for t in range(8)] + \
                            [('sf', t) for t in self_tiles[c]]
                    nst = len(steps)
                    ops_ = psout.tile([128, 512], f32, tag="out")
                    pd = psden.tile([1, 512], f32, tag="den")
                    pending = None  # (at_t, vt_l) awaiting attn@v/denominator

                    def flush(last):
                        at_p, vt_p, jj, c0p = pending
                        wp = 512 - c0p
                        nc.tensor.matmul(ops_[:, c0p:512], vt_p,
                                         at_p[:, 0:wp],
                                         start=(jj == 0), stop=last)
                        nc.tensor.matmul(pd[:, c0p:512], ones_sb[:, 0:1],
                                         at_p[:, 0:wp],
                                         start=(jj == 0), stop=last)

                    for j, (src, t) in enumerate(steps):
                        ps_s = pssc.tile([128, 512], f32, tag="sc")
                        if src == 'kb':
                            c0 = 0
                            lhsT = kbk_sb[:, 128 * t:128 * t + 128]
                            rhs = qnT[:, qcol:qcol + 512]
                            vt_l = kbv_sb[:, 128 * t:128 * t + 128]
                        else:
                            c0 = col0_map[(t, c)]
                            lhsT = kTr[:, 128 * t:128 * t + 128]
                            rhs = qTr[:, qcol + c0:qcol + 512]
                            vt_l = vkm[:, 128 * t:128 * t + 128]
                        w = 512 - c0
                        nc.tensor.matmul(ps_s[:, 0:w], lhsT, rhs,
                                         start=True, stop=True)
                        if src == 'sf' and (t, c) in mixed_idx:
                            k = mixed_idx[(t, c)]
                            nc.vector.tensor_add(
                                ps_s[:, 0:w], ps_s[:, 0:w],
                                mask_sb[:, 512 * k:512 * k + w])
                        at_t = atp.tile([128, 512], f32r, tag="at")
                        nc.scalar.activation(
                            at_t[:, 0:w], ps_s[:, 0:w],
                            mybir.ActivationFunctionType.Exp, scale=SCALE)
                        if pending is not None:
                            flush(False)
                        pending = (at_t, vt_l, j, c0)
                    flush(True)
                    den = nrm.tile([1, 512], f32, tag="den_sb")
                    nc.vector.tensor_copy(den[:], pd[:])
                    rec32 = nrm.tile([1, 512], f32, tag="rec32")
                    nc.vector.reciprocal_approx_fast(rec32[:], den[:])
                    rec = nrm.tile([1, 512], f32r, tag="rec")
                    nc.vector.tensor_copy(rec[:], rec32[:])
                    bc = psbc.tile([128, 512], f32, tag="bc")
                    nc.tensor.matmul(bc[:], ones_sb[0:1, :], rec[:],
                                     start=True, stop=True)
                    bc_sb = nrm.tile([128, 512], f32r, tag="bc_sb")
                    nc.vector.tensor_copy(bc_sb[:], bc[:])
                    nc.vector.tensor_mul(outn[:, qcol:qcol + 512],
                                         ops_[:], bc_sb[:])
                    if c == 1:
                        emit_y_tile(i)
            for st in range(4, 8):
                emit_y_tile(st)

    nc.compile()
    return nc


def kernel(hidden_states, attention_mask, position_ids, kb_keys, kb_values,
           Wq, Wq_new, Wk, Wv, Wo):
    from concourse.bass_utils import run_bass_kernel_spmd

    hidden_states = np.asarray(hidden_states, dtype=np.float32)
    attention_mask = np.asarray(attention_mask, dtype=np.float32)
    position_ids = np.asarray(position_ids)
    kb_keys = np.asarray(kb_keys, dtype=np.float32)
    kb_values = np.asarray(kb_values, dtype=np.float32)
    Wq = np.asarray(Wq, dtype=np.float32)
    Wq_new = np.asarray(Wq_new, dtype=np.float32)
    Wk = np.asarray(Wk, dtype=np.float32)
    Wv = np.asarray(Wv, dtype=np.float32)
    Wo = np.asarray(Wo, dtype=np.float32)

    # ---- host: classify self-attention mask blocks ----
    mask = attention_mask[:, 0]  # (B, S, S) [q, key]
    self_tiles = {}
    mixed = []
    col0_map = {}
    for c in range(2):
        tiles = []
        for t in range(8):
            blk = mask[:, 512 * c:512 * c + 512, 128 * t:128 * t + 128]
            if np.all(blk <= -1e8):
                continue
            tiles.append(t)
            # leading q-columns fully masked in every batch can be skipped
            colmask = np.all(blk <= -1e8, axis=(0, 2))  # (512,) per q-col
            col0 = 0
            while col0 < 512 and colmask[col0]:
                col0 += 1
            col0 = (col0 // 128) * 128  # keep 128-aligned for tidy tiles
            col0_map[(t, c)] = col0
            if np.any(blk[:, col0:, :] < 0):
                mixed.append((t, c))
        self_tiles[c] = tiles
    mixed_idx = {tc_: k for k, tc_ in enumerate(mixed)}
    n_mask = len(mixed)

    nc = _build_program(self_tiles, mixed_idx, n_mask, col0_map)

    # ---- host: shared constant prep ----
    inv_freq = 1.0 / (THETA ** (np.arange(0, HD, 2, dtype=np.float32) / HD))
    P = np.zeros((HD, HD), np.float32)
    for d in range(64):
        P[d, d + 64] = -1.0
        P[d + 64, d] = 1.0
    ropePT = np.ascontiguousarray(P.T)
    ones = np.ones((128, 128), np.float32)
    ident = np.eye(128, dtype=np.float32)

    def pack_w(wT, ndt):
        # wT (H, 128*ndt) -> (128, 2048*ndt): tile (dt) block holds 16
        # h-tiles side by side: cols 2048*dt + 128*h = wT[128h:+128, 128dt:+128]
        out = np.empty((128, 2048 * ndt), np.float32)
        for dt_i in range(ndt):
            for h in range(16):
                out[:, 2048 * dt_i + 128 * h:2048 * dt_i + 128 * h + 128] = \
                    wT[128 * h:128 * h + 128, 128 * dt_i:128 * dt_i + 128]
        return out

    cosTs, sinTs, maskTs = [], [], []
    for b in range(B):
        freqs = position_ids[b].astype(np.float32)[:, None] * inv_freq[None, :]
        emb = np.concatenate([freqs, freqs], axis=1)  # (S, 128)
        cosTs.append(np.ascontiguousarray(np.cos(emb).T.astype(np.float32)))
        sinTs.append(np.ascontiguousarray(np.sin(emb).T.astype(np.float32)))
        if n_mask:
            mt = np.zeros((128, 512 * n_mask), np.float32)
            for (t, c), k in mixed_idx.items():
                c0 = col0_map[(t, c)]
                w = 512 - c0
                mt[:, 512 * k:512 * k + w] = \
                    mask[b, 512 * c + c0:512 * c + 512,
                         128 * t:128 * t + 128].T
            maskTs.append(mt)

    in_maps = []
    for cid in range(8):
        b, g = cid // 4, cid % 4
        kbv_p = np.empty((128, KB), np.float32)
        kvb = kb_values[b, :, 128 * g:128 * g + 128]
        for t in range(8):
            kbv_p[:, 128 * t:128 * t + 128] = kvb[128 * t:128 * t + 128, :]
        wo_p = np.empty((128, 8192), np.float32)
        woT = Wo[:, 512 * g:512 * g + 512].T  # (512, 2048)
        for i in range(4):
            wo_p[:, 2048 * i:2048 * i + 2048] = woT[128 * i:128 * i + 128, :]
        m = dict(
            xT=np.ascontiguousarray(hidden_states[b].T),
            wq=pack_w(Wq[512 * g:512 * g + 512, :].T, 4),
            wqn=pack_w(Wq_new[512 * g:512 * g + 512, :].T, 4),
            wk=pack_w(Wk[128 * g:128 * g + 128, :].T, 1),
            wv=pack_w(Wv[128 * g:128 * g + 128, :].T, 1),
            wo=wo_p,
            kbkT=np.ascontiguousarray(kb_keys[b, :, 128 * g:128 * g + 128].T),
            kbv=kbv_p,
            cosT=cosTs[b], sinT=sinTs[b],
            ropePT=ropePT, ones=ones, ident=ident,
        )
        if n_mask:
            m['masks'] = maskTs[b]
        in_maps.append(m)

    res = run_bass_kernel_spmd(nc, in_maps, core_ids=list(range(8)))
    global LAST_RESULTS
    LAST_RESULTS = res

    out = np.zeros((B, S, H), np.float32)
    for cid in range(8):
        b = cid // 4
        out[b] += res.results[cid]["y"]
    return out



# revision 11
# speedup vs baseline: 1.0218x; 1.0218x over previous
"""Trainium2 Bass kernel for KBLAM Gemma3n attention (B=2, S=1024, H=2048,
NH=16, NKV=4, HD=128, KB=1024), sharded over 8 NeuronCores as
(batch x kv-head-group): core = 4*b + g handles batch b and kv head g
(which serves q-heads 4g..4g+3).  Each core computes a partial s-major
output y_part (S, H) = attn_out @ Wo[:, 512g:512g+512].T ; the host sums
the 4 partials per batch.

v2 design notes (deltas from the v1 baseline at 267us):
 - warmup matmuls on memset data to ramp the PE p-state during DMA boot.
 - DMA order tuned for time-to-first-matmul: wk split in 4, xt tiles
   h-ascending with wq0 pieces interleaved; k/v/q0 projections ride the
   xt stream (interleaved per h-tile, 6 psum banks).
 - rope / v-transpose / psum->sbuf copies pipelined behind the NEXT
   chunk's matmuls so the PE never waits on the DVE.
 - attention value path in fp16: exp writes fp16 at-tiles, v/kb_v tiles
   fp16, so narrow (128-col) moving operands run at 1 cycle/row.
 - score tiles that would be 128 cols wide are widened to 256 (f32r
   moving <256 cols runs at 1/4 rate; 256 at full rate) with the extra
   block masked via the triangle tile.
 - causal masks: one [128, 384] (neg | tri | 0) f32 tile replaces 2.1MB
   of packed per-block masks (generic packed fallback kept).
 - exp processed in [128, 1024] psum pairs (two 512-wide score tiles
   share one activation instruction) to halve Act-engine overhead.
 - y emission shares a 2-buffer psum ring with the attention output
   accumulator so the final y tiles pipeline instead of serializing.
"""
import math
from contextlib import ExitStack

import numpy as np

B, S, H = 2, 1024, 2048
NH, NKV, HD = 16, 4, 128
KB = 1024
THETA = 10000.0
SCALE = 1.0 / math.sqrt(HD)
NEG = -1e9

LAST_RESULTS = None


def _build_program(plan):
    """Build the single-core Bass/Tile program.

    plan: dict with
      self_steps: {c: [(t, col0, mask_spec), ...]} mask_spec is
        ('tri', off, w_add) or ('packed', k, w_add) or None
      n_mask: number of packed [128, 512] fallback mask tiles
      use_tri: whether the triangle tile input is present
    """
    import concourse.tile as tile
    from concourse import bacc, mybir

    f32 = mybir.dt.float32
    f32r = mybir.dt.float32r
    f16 = mybir.dt.float16
    AF = mybir.ActivationFunctionType
    nc = bacc.Bacc("TRN2", target_bir_lowering=False, debug=False,
                   enable_asserts=False, num_devices=8)

    self_steps = plan['self_steps']
    n_mask = plan['n_mask']
    use_tri = plan['use_tri']

    xT = nc.dram_tensor("xT", [H, S], f32r, kind="ExternalInput")
    # packed weights: per-dt blocks of 16 h-tiles: cols 2048*dt + 128*h
    wq = nc.dram_tensor("wq", [128, 8192], f32r, kind="ExternalInput")
    wqn = nc.dram_tensor("wqn", [128, 8192], f32r, kind="ExternalInput")
    wk = nc.dram_tensor("wk", [128, 2048], f32r, kind="ExternalInput")
    wv = nc.dram_tensor("wv", [128, 2048], f32r, kind="ExternalInput")
    # wo packed: block i at cols 2048*i = Wo_g^T[128i:128i+128, :]
    wo = nc.dram_tensor("wo", [128, 8192], f32r, kind="ExternalInput")
    kbkT = nc.dram_tensor("kbkT", [128, KB], f32r, kind="ExternalInput")
    # kbv packed key-major tiles side by side (fp16): tile t at cols 128*t
    kbv = nc.dram_tensor("kbv", [128, KB], f16, kind="ExternalInput")
    cosT = nc.dram_tensor("cosT", [128, S], f32r, kind="ExternalInput")
    sinT = nc.dram_tensor("sinT", [128, S], f32r, kind="ExternalInput")
    ropePT = nc.dram_tensor("ropePT", [128, 128], f32r, kind="ExternalInput")
    ident = nc.dram_tensor("ident", [128, 128], f32r, kind="ExternalInput")
    if use_tri:
        tri = nc.dram_tensor("tri", [128, 384], f32, kind="ExternalInput")
    if n_mask:
        masks = nc.dram_tensor("masks", [128, 512 * n_mask], f32,
                               kind="ExternalInput")
    y = nc.dram_tensor("y", [S, H], f32, kind="ExternalOutput")

    with tile.TileContext(nc) as tc, ExitStack() as ctx:
        po = ctx.enter_context(tc.tile_pool(name="projout", bufs=1))
        qTr = po.tile([128, 4096], f32r, tag="qTr")
        qnT = po.tile([128, 4096], f32r, tag="qnT")
        kTr = po.tile([128, 1024], f32r, tag="kTr")
        vkm = po.tile([128, 1024], f16, tag="vkm")
        outn = po.tile([128, 4096], f32r, tag="outn")

        consts = ctx.enter_context(tc.tile_pool(name="consts", bufs=1))
        kbp = ctx.enter_context(tc.tile_pool(name="kb", bufs=1))

        # ---------------- phase 1: projections + rope + v transpose ------
        with tc.tile_pool(name="xw", bufs=1) as xw, \
             tc.tile_pool(name="wt", bufs=3) as wpool, \
             tc.tile_pool(name="ptmp", bufs=3) as ptmp, \
             tc.tile_pool(name="psp", bufs=6, space="PSUM") as psp, \
             tc.tile_pool(name="psr", bufs=2, space="PSUM") as psr:

            # -- warmup: ramp the PE p-state while DMA boots --------------
            warm_sb = xw.tile([128, 512], f16, tag="warm")
            nc.vector.memset(warm_sb[:], 1.0)
            ones16 = consts.tile([128, 128], f16, tag="ones16")
            nc.vector.memset(ones16[:], 1.0)
            for wi in range(16):
                wps = psp.tile([128, 512], f32, tag="ck", name="wps")
                nc.tensor.matmul(wps[:], warm_sb[:, 0:128], warm_sb[:],
                                 start=True, stop=True)

            # -- DMA emission order = arrival order (FIFO over 16 engines)
            wblk_k = wpool.tile([128, 2048], f32r, tag="wblk", name="wblk_k")
            for p in range(4):
                nc.sync.dma_start(wblk_k[:, 512 * p:512 * p + 512],
                                  wk[:, 512 * p:512 * p + 512])
            wblk_v = wpool.tile([128, 2048], f32r, tag="wblk", name="wblk_v")
            for p in range(4):
                nc.sync.dma_start(wblk_v[:, 512 * p:512 * p + 512],
                                  wv[:, 512 * p:512 * p + 512])
            xt = xw.tile([128, 16384], f32r, tag="xt")
            wblk_q0 = wpool.tile([128, 2048], f32r, tag="wblk",
                                 name="wblk_q0")
            for h in range(16):
                nc.sync.dma_start(xt[:, 1024 * h:1024 * h + 1024],
                                  xT[128 * h:128 * h + 128, :])
                # wq dt0 pieces interleaved into the xt stream
                if h in (3, 6, 9, 12):
                    p = (3, 6, 9, 12).index(h)
                    nc.sync.dma_start(wblk_q0[:, 512 * p:512 * p + 512],
                                      wq[:, 512 * p:512 * p + 512])
            # remaining weights + consts, in need-order
            wblks = {('q', 0): wblk_q0}
            loads = [('qn', 0, wqn), ('q', 1, wq), ('qn', 1, wqn)]
            for kind, dt_i, src in loads:
                t_ = wpool.tile([128, 2048], f32r, tag="wblk", name="wblk")
                nc.sync.dma_start(t_[:], src[:, 2048 * dt_i:2048 * dt_i + 2048])
                wblks[(kind, dt_i)] = t_
            cos_sb = consts.tile([128, S], f32r, tag="cos")
            nc.sync.dma_start(cos_sb[:], cosT[:])
            sin_sb = consts.tile([128, S], f32r, tag="sin")
            nc.sync.dma_start(sin_sb[:], sinT[:])
            rp_sb = consts.tile([128, 128], f32r, tag="rp")
            nc.sync.dma_start(rp_sb[:], ropePT[:])
            id_sb = consts.tile([128, 128], f32r, tag="id")
            nc.sync.dma_start(id_sb[:], ident[:])
            id16 = consts.tile([128, 128], f16, tag="id16")
            nc.vector.tensor_copy(id16[:], id_sb[:])
            if use_tri:
                tri_sb = consts.tile([128, 384], f32, tag="tri")
                nc.sync.dma_start(tri_sb[:], tri[:])
            if n_mask:
                mask_sb = consts.tile([128, 512 * n_mask], f32, tag="mask")
                nc.sync.dma_start(mask_sb[:], masks[:])
            for kind, dt_i, src in [('q', 2, wq), ('qn', 2, wqn),
                                    ('q', 3, wq), ('qn', 3, wqn)]:
                t_ = wpool.tile([128, 2048], f32r, tag="wblk", name="wblk")
                nc.sync.dma_start(t_[:], src[:, 2048 * dt_i:2048 * dt_i + 2048])
                wblks[(kind, dt_i)] = t_
            kbk_sb = kbp.tile([128, KB], f32r, tag="kbk")
            nc.sync.dma_start(kbk_sb[:], kbkT[:])
            kbv_sb = kbp.tile([128, KB], f16, tag="kbv")
            nc.sync.dma_start(kbv_sb[:], kbv[:])

            vt_tmp = xw.tile([128, 1024], f16, tag="vt")

            def chunk_mms(wblk, pss):
                for h in range(16):
                    for half in range(2):
                        nc.tensor.matmul(
                            pss[half][:], wblk[:, 128 * h:128 * h + 128],
                            xt[:, 1024 * h + 512 * half:
                               1024 * h + 512 * half + 512],
                            start=(h == 0), stop=(h == 15))

            def rope_half(tmp, half, dst):
                # dst = tmp*cos + (P@tmp)*sin  (tmp already in SBUF f32r)
                pp = psr.tile([128, 512], f32, tag="aux", name="pp")
                nc.tensor.matmul(pp[:], rp_sb[:], tmp[:], start=True,
                                 stop=True)
                cs = cos_sb[:, 512 * half:512 * half + 512]
                sn = sin_sb[:, 512 * half:512 * half + 512]
                nc.vector.tensor_mul(dst, tmp[:], cs)
                tmp2 = ptmp.tile([128, 512], f32r, tag="tmp2")
                nc.vector.tensor_mul(tmp2[:], pp[:], sn)
                nc.vector.tensor_add(dst, dst, tmp2[:])

            # --- interleaved triple (k, v, q0) riding the xt stream ------
            pk = [psp.tile([128, 512], f32, tag="ck", name="pk")
                  for _ in range(2)]
            pv = [psp.tile([128, 512], f32, tag="ck", name="pv")
                  for _ in range(2)]
            pq0 = [psp.tile([128, 512], f32, tag="ck", name="pq0")
                   for _ in range(2)]
            for h in range(16):
                for half in range(2):
                    nc.tensor.matmul(
                        pk[half][:], wblk_k[:, 128 * h:128 * h + 128],
                        xt[:, 1024 * h + 512 * half:1024 * h + 512 * half + 512],
                        start=(h == 0), stop=(h == 15))
                for half in range(2):
                    nc.tensor.matmul(
                        pv[half][:], wblk_v[:, 128 * h:128 * h + 128],
                        xt[:, 1024 * h + 512 * half:1024 * h + 512 * half + 512],
                        start=(h == 0), stop=(h == 15))
                if h >= 4:
                    hh = h - 4
                    for half in range(2):
                        nc.tensor.matmul(
                            pq0[half][:], wblk_q0[:, 128 * hh:128 * hh + 128],
                            xt[:, 1024 * hh + 512 * half:
                               1024 * hh + 512 * half + 512],
                            start=(hh == 0), stop=False)
            for hh in range(12, 16):
                for half in range(2):
                    nc.tensor.matmul(
                        pq0[half][:], wblk_q0[:, 128 * hh:128 * hh + 128],
                        xt[:, 1024 * hh + 512 * half:
                           1024 * hh + 512 * half + 512],
                        start=False, stop=(hh == 15))

            # psum -> sbuf copies (DVE) free the 6 banks for later chunks
            tmps = {}
            for nm, pp_ in (('k0', pk[0]), ('k1', pk[1]),
                            ('q00', pq0[0]), ('q01', pq0[1])):
                t_ = ptmp.tile([128, 512], f32r, tag="tmp", name="t" + nm,
                               bufs=6)
                nc.vector.tensor_copy(t_[:], pp_[:])
                tmps[nm] = t_
            for half in range(2):
                nc.vector.tensor_copy(
                    vt_tmp[:, 512 * half:512 * half + 512], pv[half][:])

            # v transposes (fp16): fill the gap until wqn0 arrives
            for t in range(8):
                pst = psr.tile([128, 128], f16, tag="aux", name="ptr")
                nc.tensor.transpose(
                    pst[:], vt_tmp[:, 128 * t:128 * t + 128], id16[:])
                nc.vector.tensor_copy(vkm[:, 128 * t:128 * t + 128], pst[:])

            # --- remaining chunks, post-processing pipelined one behind --
            # order: qn0, q1, qn1, q2, qn2, q3, qn3
            post_q = []  # deferred rope/copy closures

            def post_k():
                for half in range(2):
                    rope_half(tmps['k' + str(half)], half,
                              kTr[:, 512 * half:512 * half + 512])

            def mk_post_q(tm0, tm1, dt_i):
                def f():
                    rope_half(tm0, 0, qTr[:, 1024 * dt_i:1024 * dt_i + 512])
                    rope_half(tm1, 1,
                              qTr[:, 1024 * dt_i + 512:1024 * dt_i + 1024])
                return f

            def mk_post_qn(p0, p1, dt_i):
                def f():
                    nc.vector.tensor_copy(
                        qnT[:, 1024 * dt_i:1024 * dt_i + 512], p0[:])
                    nc.scalar.activation(
                        qnT[:, 1024 * dt_i + 512:1024 * dt_i + 1024], p1[:],
                        AF.Copy)
                return f

            post_q.append(post_k)
            post_q.append(mk_post_q(tmps['q00'], tmps['q01'], 0))

            rest = [('qn', 0), ('q', 1), ('qn', 1), ('q', 2), ('qn', 2),
                    ('q', 3), ('qn', 3)]
            for kind, dt_i in rest:
                pss = [psp.tile([128, 512], f32, tag="ck", name="pc0"),
                       psp.tile([128, 512], f32, tag="ck", name="pc1")]
                chunk_mms(wblks[(kind, dt_i)], pss)
                # run one deferred post-processing batch behind these mms
                if post_q:
                    post_q.pop(0)()
                if kind == 'q':
                    tm0 = ptmp.tile([128, 512], f32r, tag="tmp", name="tmq0",
                                    bufs=6)
                    nc.vector.tensor_copy(tm0[:], pss[0][:])
                    tm1 = ptmp.tile([128, 512], f32r, tag="tmp", name="tmq1",
                                    bufs=6)
                    nc.vector.tensor_copy(tm1[:], pss[1][:])
                    post_q.append(mk_post_q(tm0, tm1, dt_i))
                else:
                    post_q.append(mk_post_qn(pss[0], pss[1], dt_i))
            while post_q:
                post_q.pop(0)()

        # ---------------- phase 2: attention ------------------------------
        onp = ctx.enter_context(tc.tile_pool(name="onp", bufs=1))
        wo_sb = onp.tile([128, 8192], f32r, tag="wo")
        nc.sync.dma_start(wo_sb[:], wo[:])

        with tc.tile_pool(name="at", bufs=4) as atp, \
             tc.tile_pool(name="nrm", bufs=3) as nrm, \
             tc.tile_pool(name="pssc", bufs=2, space="PSUM") as pssc, \
             tc.tile_pool(name="psout", bufs=2, space="PSUM") as psout, \
             tc.tile_pool(name="psden", bufs=1, space="PSUM") as psden, \
             tc.tile_pool(name="psbc", bufs=1, space="PSUM") as psbc, \
             tc.tile_pool(name="ysb", bufs=2) as ysbp:

            ncopy = [0]

            def psum_copy(dst, src):
                # alternate DVE / Act for psum->sbuf copies
                ncopy[0] += 1
                if ncopy[0] % 2:
                    nc.vector.tensor_copy(dst, src)
                else:
                    nc.scalar.activation(dst, src, AF.Copy)

            def emit_y_tile(st):
                cy, off = st // 4, 128 * (st % 4)
                ysb = ysbp.tile([128, 2048], f32, tag="ysb", name="ysb")
                for n in range(4):
                    py = psout.tile([128, 512], f32, tag="out", name="py")
                    for i in range(4):
                        lcol = 1024 * i + 512 * cy + off
                        nc.tensor.matmul(
                            py[:], outn[:, lcol:lcol + 128],
                            wo_sb[:, 2048 * i + 512 * n:
                                  2048 * i + 512 * n + 512],
                            start=(i == 0), stop=(i == 3))
                    psum_copy(ysb[:, 512 * n:512 * n + 512], py[:])
                nc.sync.dma_start(y[128 * st:128 * st + 128, :], ysb[:])

            deferred = [None]  # previous head's normalize(+emit) closure

            for c in range(2):
                for i in range(4):
                    qcol = 1024 * i + 512 * c
                    # steps: (src, t, col0, wsc, mask_spec, widened)
                    steps = [('kb', t, 0, 512, None, False) for t in range(8)]
                    for (t, col0, mspec, widened) in self_steps[c]:
                        steps.append(('sf', t, col0, 512 - col0, mspec,
                                      widened))
                    nst = len(steps)
                    # ops_/pd allocated lazily at the first flush: the "out"
                    # psum ring is shared with emit_y_tile's py tiles, and
                    # the deferred previous-head emit must claim its ring
                    # slots BEFORE this head's accumulator does.
                    acc = {}

                    def get_acc(acc=acc):
                        if 'ops' not in acc:
                            acc['ops'] = psout.tile([128, 512], f32,
                                                    tag="out", name="ops")
                            acc['pd'] = psden.tile([1, 512], f32, tag="den",
                                                   name="pd")
                        return acc['ops'], acc['pd']

                    # group steps into exp units: pairs of 512-wide steps,
                    # singles otherwise
                    units = []
                    j = 0
                    while j < nst:
                        if (j + 1 < nst and steps[j][3] == 512
                                and steps[j + 1][3] == 512):
                            units.append((j, j + 1))
                            j += 2
                        else:
                            units.append((j,))
                            j += 1

                    pending = []  # av/den jobs

                    def flush(jobs, get_acc=get_acc, nst=nst):
                        ops_, pd = get_acc()
                        for (at_sb, atoff, vt_l, c0av, wav, jidx) in jobs:
                            first = (jidx == 0)
                            last = (jidx == nst - 1)
                            nc.tensor.matmul(
                                ops_[:, c0av:c0av + wav], vt_l,
                                at_sb[:, atoff:atoff + wav],
                                start=first, stop=last)
                            nc.tensor.matmul(
                                pd[:, c0av:c0av + wav], ones16[:, 0:1],
                                at_sb[:, atoff:atoff + wav],
                                start=first, stop=last)

                    for ui, unit in enumerate(units):
                        sc = pssc.tile([128, 1024], f32, tag="sc", name="sc")
                        at_t = atp.tile([128, 1024], f16, tag="at",
                                        name="at")
                        jobs = []
                        off = 0
                        for j in unit:
                            src, t, col0, wsc, mspec, widened = steps[j]
                            if src == 'kb':
                                lhsT = kbk_sb[:, 128 * t:128 * t + 128]
                                rhs = qnT[:, qcol:qcol + 512]
                                vt_l = kbv_sb[:, 128 * t:128 * t + 128]
                            else:
                                lhsT = kTr[:, 128 * t:128 * t + 128]
                                rhs = qTr[:, qcol + col0:qcol + 512]
                                vt_l = vkm[:, 128 * t:128 * t + 128]
                            nc.tensor.matmul(sc[:, off:off + wsc], lhsT, rhs,
                                             start=True, stop=True)
                            if mspec is not None and mspec[0] == 'tri':
                                _, toff, wadd = mspec
                                nc.vector.tensor_add(
                                    sc[:, off:off + wadd],
                                    sc[:, off:off + wadd],
                                    tri_sb[:, toff:toff + wadd])
                            elif mspec is not None and mspec[0] == 'packed':
                                _, kidx, wadd = mspec
                                nc.vector.tensor_add(
                                    sc[:, off:off + wadd],
                                    sc[:, off:off + wadd],
                                    mask_sb[:, 512 * kidx:512 * kidx + wadd])
                            skip = 128 if widened else 0
                            jobs.append((at_t, off + skip, vt_l,
                                         col0 + skip, wsc - skip, j))
                            off += wsc
                        nc.scalar.activation(at_t[:, 0:off], sc[:, 0:off],
                                             AF.Exp, scale=SCALE)
                        if ui == 0 and deferred[0] is not None:
                            # run previous head's normalize/emit now: its
                            # DVE reciprocal chain overlaps our scores
                            deferred[0]()
                            deferred[0] = None
                        if pending:
                            flush(pending)
                        pending = jobs
                    flush(pending)
                    ops_, pd = get_acc()

                    def normalize(ops_=ops_, pd=pd, qcol=qcol, c=c, i=i):
                        rec32 = nrm.tile([1, 512], f32, tag="rec32")
                        nc.vector.reciprocal_approx_fast(rec32[:], pd[:])
                        rec = nrm.tile([1, 512], f16, tag="rec")
                        nc.vector.tensor_copy(rec[:], rec32[:])
                        bc = psbc.tile([128, 512], f32, tag="bc")
                        nc.tensor.matmul(bc[:], ones16[0:1, :], rec[:],
                                         start=True, stop=True)
                        bc_sb = nrm.tile([128, 512], f32r, tag="bc_sb")
                        nc.vector.tensor_copy(bc_sb[:], bc[:])
                        nc.vector.tensor_mul(outn[:, qcol:qcol + 512],
                                             ops_[:], bc_sb[:])
                        if c == 1:
                            emit_y_tile(i)

                    deferred[0] = normalize
            deferred[0]()
            for st in range(4, 8):
                emit_y_tile(st)

    nc.compile()
    return nc


def kernel(hidden_states, attention_mask, position_ids, kb_keys, kb_values,
           Wq, Wq_new, Wk, Wv, Wo):
    from concourse.bass_utils import run_bass_kernel_spmd

    hidden_states = np.asarray(hidden_states, dtype=np.float32)
    attention_mask = np.asarray(attention_mask, dtype=np.float32)
    position_ids = np.asarray(position_ids)
    kb_keys = np.asarray(kb_keys, dtype=np.float32)
    kb_values = np.asarray(kb_values, dtype=np.float32)
    Wq = np.asarray(Wq, dtype=np.float32)
    Wq_new = np.asarray(Wq_new, dtype=np.float32)
    Wk = np.asarray(Wk, dtype=np.float32)
    Wv = np.asarray(Wv, dtype=np.float32)
    Wo = np.asarray(Wo, dtype=np.float32)

    # ---- host: classify self-attention mask blocks ----
    mask = attention_mask[:, 0]  # (B, S, S) [q, key]
    tri_blk = np.where(
        np.arange(128)[None, :] >= np.arange(128)[:, None], 0.0,
        NEG).astype(np.float32)  # [key, q] triangle

    self_steps = {}
    packed = []  # (b-independent) packed fallback mask blocks, [key, q]
    use_tri = False
    for c in range(2):
        lst = []
        for t in range(8):
            blk = mask[:, 512 * c:512 * c + 512, 128 * t:128 * t + 128]
            if np.all(blk <= -1e8):
                continue
            colmask = np.all(blk <= -1e8, axis=(0, 2))  # (512,) per q-col
            col0 = 0
            while col0 < 512 and colmask[col0]:
                col0 += 1
            col0 = (col0 // 128) * 128
            sub = blk[:, col0:, :]  # (B, w, 128) [q, key]
            if not np.any(sub < 0):
                lst.append((t, col0, None, False))
                continue
            # mixed: is it the canonical causal triangle at window start?
            w = 512 - col0
            exp_pat = np.zeros((w, 128), np.float32)
            exp_pat[:128] = tri_blk.T  # [q, key]
            is_tri = all(np.array_equal(sub[b_], exp_pat) for b_ in range(B))
            if is_tri:
                use_tri = True
                if w == 128:
                    # widen to 256 (f32r <256-col moving runs at 1/4 rate);
                    # extra leading block is fully masked, av/den skip it
                    lst.append((t, col0 - 128, ('tri', 0, 256), True))
                else:
                    lst.append((t, col0, ('tri', 128, 128), False))
            else:
                packed.append((c, t, col0))
                lst.append((t, col0, ('packed', len(packed) - 1, w), False))
        self_steps[c] = lst
    n_mask = len(packed)

    plan = dict(self_steps=self_steps, n_mask=n_mask, use_tri=use_tri)
    nc = _build_program(plan)

    # ---- host: shared constant prep ----
    inv_freq = 1.0 / (THETA ** (np.arange(0, HD, 2, dtype=np.float32) / HD))
    P = np.zeros((HD, HD), np.float32)
    for d in range(64):
        P[d, d + 64] = -1.0
        P[d + 64, d] = 1.0
    ropePT = np.ascontiguousarray(P.T)
    ident = np.eye(128, dtype=np.float32)
    tri384 = np.concatenate([np.full((128, 128), NEG, np.float32),
                             tri_blk, np.zeros((128, 128), np.float32)],
                            axis=1)

    def pack_w(wT, ndt):
        # wT (H, 128*ndt) -> (128, 2048*ndt): tile (dt) block holds 16
        # h-tiles side by side: cols 2048*dt + 128*h = wT[128h:+128, 128dt:+128]
        out = np.empty((128, 2048 * ndt), np.float32)
        for dt_i in range(ndt):
            for h in range(16):
                out[:, 2048 * dt_i + 128 * h:2048 * dt_i + 128 * h + 128] = \
                    wT[128 * h:128 * h + 128, 128 * dt_i:128 * dt_i + 128]
        return out

    cosTs, sinTs, maskTs = [], [], []
    for b in range(B):
        freqs = position_ids[b].astype(np.float32)[:, None] * inv_freq[None, :]
        emb = np.concatenate([freqs, freqs], axis=1)  # (S, 128)
        cosTs.append(np.ascontiguousarray(np.cos(emb).T.astype(np.float32)))
        sinTs.append(np.ascontiguousarray(np.sin(emb).T.astype(np.float32)))
        if n_mask:
            mt = np.zeros((128, 512 * n_mask), np.float32)
            for kidx, (c, t, col0) in enumerate(packed):
                w = 512 - col0
                mt[:, 512 * kidx:512 * kidx + w] = \
                    mask[b, 512 * c + col0:512 * c + 512,
                         128 * t:128 * t + 128].T
            maskTs.append(mt)

    in_maps = []
    for cid in range(8):
        b, g = cid // 4, cid % 4
        kbv_p = np.empty((128, KB), np.float16)
        kvb = kb_values[b, :, 128 * g:128 * g + 128]
        for t in range(8):
            kbv_p[:, 128 * t:128 * t + 128] = \
                kvb[128 * t:128 * t + 128, :].astype(np.float16)
        wo_p = np.empty((128, 8192), np.float32)
        woT = Wo[:, 512 * g:512 * g + 512].T  # (512, 2048)
        for i in range(4):
            wo_p[:, 2048 * i:2048 * i + 2048] = woT[128 * i:128 * i + 128, :]
        m = dict(
            xT=np.ascontiguousarray(hidden_states[b].T),
            wq=pack_w(Wq[512 * g:512 * g + 512, :].T, 4),
            wqn=pack_w(Wq_new[512 * g:512 * g + 512, :].T, 4),
            wk=pack_w(Wk[128 * g:128 * g + 128, :].T, 1),
            wv=pack_w(Wv[128 * g:128 * g + 128, :].T, 1),
            wo=wo_p,
            kbkT=np.ascontiguousarray(kb_keys[b, :, 128 * g:128 * g + 128].T),
            kbv=kbv_p,
            cosT=cosTs[b], sinT=sinTs[b],
            ropePT=ropePT, ident=ident,
        )
        if use_tri:
            m['tri'] = tri384
        if n_mask:
            m['masks'] = maskTs[b]
        in_maps.append(m)

    res = run_bass_kernel_spmd(nc, in_maps, core_ids=list(range(8)))
    global LAST_RESULTS
    LAST_RESULTS = res

    out = np.zeros((B, S, H), np.float32)
    for cid in range(8):
        b = cid // 4
        out[b] += res.results[cid]["y"]
    return out


# revision 12
# speedup vs baseline: 1.0834x; 1.0603x over previous
"""Trainium2 Bass kernel for KBLAM Gemma3n attention (B=2, S=1024, H=2048,
NH=16, NKV=4, HD=128, KB=1024), sharded over 8 NeuronCores as
(batch x kv-head-group): core = 4*b + g handles batch b and kv head g
(which serves q-heads 4g..4g+3).  Each core computes a partial s-major
output y_part (S, H) = attn_out @ Wo[:, 512g:512g+512].T ; the host sums
the 4 partials per batch.

v3 design notes (v1 baseline 267us; v2 fp16 experiment showed fp16
ldweights serialize with fp16 matmuls, so the value path stays f32r):
 - fp16 warmup matmuls on memset data ramp the PE p-state during DMA boot.
 - DMA order tuned for time-to-first-matmul: wk split in 2, then the xt
   stream with wq0 pieces interleaved; k/v/q0 projections ride the xt
   stream (interleaved per h-tile, 6 psum banks).
 - rope / v-transpose / psum->sbuf copies pipelined behind the NEXT
   chunk's matmuls so the PE never waits on the DVE.
 - score tiles that would be 128 cols wide are widened to 256 (f32r
   moving <256 cols runs at 1/4 rate; 256 at full rate) with the extra
   block masked via the triangle tile; av/den include the zeroed block.
 - causal masks: one [128, 384] (neg | tri | 0) f32 tile replaces 2.1MB
   of packed per-block masks (generic packed fallback kept).
 - exp processed in [128, 1024] psum pairs (two 512-wide score tiles
   share one activation instruction) to halve Act-engine overhead.
 - per-head normalization (reciprocal chain + y emit) deferred into the
   next head's first attention unit so the PE never waits on the DVE.
 - y tiles 4-7: heads 0-2 partial sums emitted during head 3's
   attention; only the head-3 contribution + add + quadrant DMA remain
   in the tail.
"""
import math
from contextlib import ExitStack

import numpy as np

B, S, H = 2, 1024, 2048
NH, NKV, HD = 16, 4, 128
KB = 1024
THETA = 10000.0
SCALE = 1.0 / math.sqrt(HD)
NEG = -1e9

LAST_RESULTS = None


def _build_program(plan):
    """Build the single-core Bass/Tile program.

    plan: dict with
      self_steps: {c: [(t, col0, mask_spec, widened), ...]} mask_spec is
        ('tri', off, w_add) or ('packed', k, w_add) or None
      n_mask: number of packed [128, 512] fallback mask tiles
      use_tri: whether the triangle tile input is present
    """
    import concourse.tile as tile
    from concourse import bacc, mybir

    f32 = mybir.dt.float32
    f32r = mybir.dt.float32r
    f16 = mybir.dt.float16
    AF = mybir.ActivationFunctionType
    nc = bacc.Bacc("TRN2", target_bir_lowering=False, debug=False,
                   enable_asserts=False, num_devices=8)

    self_steps = plan['self_steps']
    n_mask = plan['n_mask']
    use_tri = plan['use_tri']

    xT = nc.dram_tensor("xT", [H, S], f32r, kind="ExternalInput")
    # packed weights: per-dt blocks of 16 h-tiles: cols 2048*dt + 128*h
    wq = nc.dram_tensor("wq", [128, 8192], f32r, kind="ExternalInput")
    wqn = nc.dram_tensor("wqn", [128, 8192], f32r, kind="ExternalInput")
    wk = nc.dram_tensor("wk", [128, 2048], f32r, kind="ExternalInput")
    wv = nc.dram_tensor("wv", [128, 2048], f32r, kind="ExternalInput")
    # wo packed: block i at cols 2048*i = Wo_g^T[128i:128i+128, :]
    wo = nc.dram_tensor("wo", [128, 8192], f32r, kind="ExternalInput")
    kbkT = nc.dram_tensor("kbkT", [128, KB], f32r, kind="ExternalInput")
    # kbv packed key-major tiles side by side: tile t at cols 128*t
    kbv = nc.dram_tensor("kbv", [128, KB], f32r, kind="ExternalInput")
    cosT = nc.dram_tensor("cosT", [128, S], f32r, kind="ExternalInput")
    sinT = nc.dram_tensor("sinT", [128, S], f32r, kind="ExternalInput")
    ropePT = nc.dram_tensor("ropePT", [128, 128], f32r, kind="ExternalInput")
    ident = nc.dram_tensor("ident", [128, 128], f32r, kind="ExternalInput")
    ones = nc.dram_tensor("ones", [128, 128], f32r, kind="ExternalInput")
    if use_tri:
        tri = nc.dram_tensor("tri", [128, 384], f32, kind="ExternalInput")
    if n_mask:
        masks = nc.dram_tensor("masks", [128, 512 * n_mask], f32,
                               kind="ExternalInput")
    y = nc.dram_tensor("y", [S, H], f32, kind="ExternalOutput")

    with tile.TileContext(nc) as tc, ExitStack() as ctx:
        po = ctx.enter_context(tc.tile_pool(name="projout", bufs=1))
        qTr = po.tile([128, 4096], f32r, tag="qTr")
        qnT = po.tile([128, 4096], f32r, tag="qnT")
        kTr = po.tile([128, 1024], f32r, tag="kTr")
        vkm = po.tile([128, 1024], f32r, tag="vkm")
        outn = po.tile([128, 4096], f32r, tag="outn")

        consts = ctx.enter_context(tc.tile_pool(name="consts", bufs=1))
        kbp = ctx.enter_context(tc.tile_pool(name="kb", bufs=1))

        # ---------------- phase 1: projections + rope + v transpose ------
        with tc.tile_pool(name="xw", bufs=1) as xw, \
             tc.tile_pool(name="wt", bufs=5) as wpool, \
             tc.tile_pool(name="ptmp", bufs=3) as ptmp, \
             tc.tile_pool(name="psp", bufs=6, space="PSUM") as psp, \
             tc.tile_pool(name="psr", bufs=2, space="PSUM") as psr:

            # -- warmup: ramp the PE p-state while DMA boots --------------
            warm_sb = xw.tile([128, 512], f16, tag="warm")
            nc.vector.memset(warm_sb[:], 1.0)
            for wi in range(24):
                wps = psp.tile([128, 512], f32, tag="ck", name="wps")
                nc.tensor.matmul(wps[:], warm_sb[:, 0:128], warm_sb[:],
                                 start=True, stop=True)

            # -- DMA emission order = arrival order (FIFO over 16 engines)
            wblk_k = wpool.tile([128, 2048], f32r, tag="wblk", name="wblk_k")
            for p in range(2):
                nc.sync.dma_start(wblk_k[:, 1024 * p:1024 * p + 1024],
                                  wk[:, 1024 * p:1024 * p + 1024])
            wblk_v = wpool.tile([128, 2048], f32r, tag="wblk", name="wblk_v")
            nc.sync.dma_start(wblk_v[:], wv[:])
            xt = xw.tile([128, 16384], f32r, tag="xt")
            wblk_q0 = wpool.tile([128, 2048], f32r, tag="wblk",
                                 name="wblk_q0")
            for h in range(16):
                nc.sync.dma_start(xt[:, 1024 * h:1024 * h + 1024],
                                  xT[128 * h:128 * h + 128, :])
                # wq dt0 halves interleaved into the xt stream
                if h in (3, 5):
                    p = (3, 5).index(h)
                    nc.sync.dma_start(wblk_q0[:, 1024 * p:1024 * p + 1024],
                                      wq[:, 1024 * p:1024 * p + 1024])
            # remaining weights + consts, in need-order
            wblks = {('q', 0): wblk_q0}
            loads = [('qn', 0, wqn), ('q', 1, wq), ('qn', 1, wqn)]
            for kind, dt_i, src in loads:
                t_ = wpool.tile([128, 2048], f32r, tag="wblk", name="wblk")
                nc.sync.dma_start(t_[:], src[:, 2048 * dt_i:2048 * dt_i + 2048])
                wblks[(kind, dt_i)] = t_
            cos_sb = consts.tile([128, S], f32r, tag="cos")
            nc.sync.dma_start(cos_sb[:], cosT[:])
            sin_sb = consts.tile([128, S], f32r, tag="sin")
            nc.sync.dma_start(sin_sb[:], sinT[:])
            rp_sb = consts.tile([128, 128], f32r, tag="rp")
            nc.sync.dma_start(rp_sb[:], ropePT[:])
            id_sb = consts.tile([128, 128], f32r, tag="id")
            nc.sync.dma_start(id_sb[:], ident[:])
            ones_sb = consts.tile([128, 128], f32r, tag="ones")
            nc.sync.dma_start(ones_sb[:], ones[:])
            if use_tri:
                tri_sb = consts.tile([128, 384], f32, tag="tri")
                nc.sync.dma_start(tri_sb[:], tri[:])
            if n_mask:
                mask_sb = consts.tile([128, 512 * n_mask], f32, tag="mask")
                nc.sync.dma_start(mask_sb[:], masks[:])
            for kind, dt_i, src in [('q', 2, wq), ('qn', 2, wqn),
                                    ('q', 3, wq), ('qn', 3, wqn)]:
                t_ = wpool.tile([128, 2048], f32r, tag="wblk", name="wblk")
                nc.sync.dma_start(t_[:], src[:, 2048 * dt_i:2048 * dt_i + 2048])
                wblks[(kind, dt_i)] = t_
            kbk_sb = kbp.tile([128, KB], f32r, tag="kbk")
            nc.sync.dma_start(kbk_sb[:], kbkT[:])
            kbv_sb = kbp.tile([128, KB], f32r, tag="kbv")
            nc.sync.dma_start(kbv_sb[:], kbv[:])

            vt_tmp = xw.tile([128, 1024], f32r, tag="vt")

            def chunk_mms(wblk, pss):
                for h in range(16):
                    for half in range(2):
                        nc.tensor.matmul(
                            pss[half][:], wblk[:, 128 * h:128 * h + 128],
                            xt[:, 1024 * h + 512 * half:
                               1024 * h + 512 * half + 512],
                            start=(h == 0), stop=(h == 15))

            def rope_half(tmp, half, dst):
                # dst = tmp*cos + (P@tmp)*sin  (tmp already in SBUF f32r)
                pp = psr.tile([128, 512], f32, tag="aux", name="pp")
                nc.tensor.matmul(pp[:], rp_sb[:], tmp[:], start=True,
                                 stop=True)
                cs = cos_sb[:, 512 * half:512 * half + 512]
                sn = sin_sb[:, 512 * half:512 * half + 512]
                nc.vector.tensor_mul(dst, tmp[:], cs)
                tmp2 = ptmp.tile([128, 512], f32r, tag="tmp2")
                nc.vector.tensor_mul(tmp2[:], pp[:], sn)
                nc.vector.tensor_add(dst, dst, tmp2[:])

            # --- interleaved triple (k, v, q0) riding the xt stream ------
            pk = [psp.tile([128, 512], f32, tag="ck", name="pk")
                  for _ in range(2)]
            pv = [psp.tile([128, 512], f32, tag="ck", name="pv")
                  for _ in range(2)]
            pq0 = [psp.tile([128, 512], f32, tag="ck", name="pq0")
                   for _ in range(2)]
            for h in range(16):
                for half in range(2):
                    nc.tensor.matmul(
                        pk[half][:], wblk_k[:, 128 * h:128 * h + 128],
                        xt[:, 1024 * h + 512 * half:1024 * h + 512 * half + 512],
                        start=(h == 0), stop=(h == 15))
                for half in range(2):
                    nc.tensor.matmul(
                        pv[half][:], wblk_v[:, 128 * h:128 * h + 128],
                        xt[:, 1024 * h + 512 * half:1024 * h + 512 * half + 512],
                        start=(h == 0), stop=(h == 15))
                if h >= 6:
                    hh = h - 6
                    for half in range(2):
                        nc.tensor.matmul(
                            pq0[half][:], wblk_q0[:, 128 * hh:128 * hh + 128],
                            xt[:, 1024 * hh + 512 * half:
                               1024 * hh + 512 * half + 512],
                            start=(hh == 0), stop=False)
            for hh in range(10, 16):
                for half in range(2):
                    nc.tensor.matmul(
                        pq0[half][:], wblk_q0[:, 128 * hh:128 * hh + 128],
                        xt[:, 1024 * hh + 512 * half:
                           1024 * hh + 512 * half + 512],
                        start=False, stop=(hh == 15))

            # psum -> sbuf copies (DVE) free the 6 banks for later chunks
            tmps = {}
            for nm, pp_ in (('k0', pk[0]), ('k1', pk[1]),
                            ('q00', pq0[0]), ('q01', pq0[1])):
                t_ = ptmp.tile([128, 512], f32r, tag="tmp", name="t" + nm,
                               bufs=6)
                nc.vector.tensor_copy(t_[:], pp_[:])
                tmps[nm] = t_
            for half in range(2):
                nc.vector.tensor_copy(
                    vt_tmp[:, 512 * half:512 * half + 512], pv[half][:])

            # v transposes: fill the gap until wqn0 arrives
            for t in range(8):
                pst = psr.tile([128, 128], f32r, tag="aux", name="ptr")
                nc.tensor.transpose(
                    pst[:], vt_tmp[:, 128 * t:128 * t + 128], id_sb[:])
                nc.vector.tensor_copy(vkm[:, 128 * t:128 * t + 128], pst[:])

            # --- remaining chunks, post-processing pipelined one behind --
            post_q = []  # deferred rope/copy closures

            def post_k():
                for half in range(2):
                    rope_half(tmps['k' + str(half)], half,
                              kTr[:, 512 * half:512 * half + 512])

            def mk_post_q(tm0, tm1, dt_i):
                def f():
                    rope_half(tm0, 0, qTr[:, 1024 * dt_i:1024 * dt_i + 512])
                    rope_half(tm1, 1,
                              qTr[:, 1024 * dt_i + 512:1024 * dt_i + 1024])
                return f

            def mk_post_qn(p0, p1, dt_i):
                def f():
                    nc.vector.tensor_copy(
                        qnT[:, 1024 * dt_i:1024 * dt_i + 512], p0[:])
                    nc.scalar.activation(
                        qnT[:, 1024 * dt_i + 512:1024 * dt_i + 1024], p1[:],
                        AF.Copy)
                return f

            post_q.append(post_k)
            post_q.append(mk_post_q(tmps['q00'], tmps['q01'], 0))

            rest = [('qn', 0), ('q', 1), ('qn', 1), ('q', 2), ('qn', 2),
                    ('q', 3), ('qn', 3)]
            for kind, dt_i in rest:
                pss = [psp.tile([128, 512], f32, tag="ck", name="pc0"),
                       psp.tile([128, 512], f32, tag="ck", name="pc1")]
                chunk_mms(wblks[(kind, dt_i)], pss)
                # run one deferred post-processing batch behind these mms
                if post_q:
                    post_q.pop(0)()
                if kind == 'q':
                    tm0 = ptmp.tile([128, 512], f32r, tag="tmp", name="tmq0",
                                    bufs=6)
                    nc.vector.tensor_copy(tm0[:], pss[0][:])
                    tm1 = ptmp.tile([128, 512], f32r, tag="tmp", name="tmq1",
                                    bufs=6)
                    nc.vector.tensor_copy(tm1[:], pss[1][:])
                    post_q.append(mk_post_q(tm0, tm1, dt_i))
                else:
                    post_q.append(mk_post_qn(pss[0], pss[1], dt_i))
            while post_q:
                post_q.pop(0)()

        # ---------------- phase 2: attention ------------------------------
        onp = ctx.enter_context(tc.tile_pool(name="onp", bufs=1))
        wo_sb = onp.tile([128, 8192], f32r, tag="wo")
        nc.sync.dma_start(wo_sb[:], wo[:])

        with tc.tile_pool(name="at", bufs=4) as atp, \
             tc.tile_pool(name="nrm", bufs=3) as nrm, \
             tc.tile_pool(name="pssc", bufs=2, space="PSUM") as pssc, \
             tc.tile_pool(name="psout", bufs=2, space="PSUM") as psout, \
             tc.tile_pool(name="psden", bufs=1, space="PSUM") as psden, \
             tc.tile_pool(name="psbc", bufs=1, space="PSUM") as psbc, \
             tc.tile_pool(name="ysb", bufs=2) as ysbp:

            ncopy = [0]

            def psum_copy(dst, src):
                # alternate DVE / Act for psum->sbuf copies
                ncopy[0] += 1
                if ncopy[0] % 2:
                    nc.vector.tensor_copy(dst, src)
                else:
                    nc.scalar.activation(dst, src, AF.Copy)

            def emit_y_tile(st):
                # full 4-head y tile (used for st 0-3, inline during c=1)
                cy, off = st // 4, 128 * (st % 4)
                ysb = ysbp.tile([128, 2048], f32, tag="ysb", name="ysb")
                for n in range(4):
                    py = psout.tile([128, 512], f32, tag="out", name="py")
                    for i in range(4):
                        lcol = 1024 * i + 512 * cy + off
                        nc.tensor.matmul(
                            py[:], outn[:, lcol:lcol + 128],
                            wo_sb[:, 2048 * i + 512 * n:
                                  2048 * i + 512 * n + 512],
                            start=(i == 0), stop=(i == 3))
                    psum_copy(ysb[:, 512 * n:512 * n + 512], py[:])
                nc.sync.dma_start(y[128 * st:128 * st + 128, :], ysb[:])

            ysb2 = {}

            def emit_y_partial(st):
                # heads 0-2 partial for y tile st (st 4-7), into ysb2[st]
                off = 128 * (st % 4)
                ysb = ysbp.tile([128, 2048], f32, tag="ysb2", name="ysb2",
                                bufs=4)
                ysb2[st] = ysb
                for n in range(4):
                    py = psout.tile([128, 512], f32, tag="out", name="pyp")
                    for i in range(3):
                        lcol = 1024 * i + 512 + off
                        nc.tensor.matmul(
                            py[:], outn[:, lcol:lcol + 128],
                            wo_sb[:, 2048 * i + 512 * n:
                                  2048 * i + 512 * n + 512],
                            start=(i == 0), stop=(i == 2))
                    psum_copy(ysb[:, 512 * n:512 * n + 512], py[:])

            def emit_y_final(st):
                # head-3 contribution + add + per-quadrant DMA
                off = 128 * (st % 4)
                ysb = ysb2[st]
                lcol = 1024 * 3 + 512 + off
                for n in range(4):
                    py = psout.tile([128, 512], f32, tag="out", name="pyf")
                    nc.tensor.matmul(
                        py[:], outn[:, lcol:lcol + 128],
                        wo_sb[:, 2048 * 3 + 512 * n:2048 * 3 + 512 * n + 512],
                        start=True, stop=True)
                    nc.vector.tensor_add(ysb[:, 512 * n:512 * n + 512],
                                         ysb[:, 512 * n:512 * n + 512],
                                         py[:])
                    nc.sync.dma_start(
                        y[128 * st:128 * st + 128, 512 * n:512 * n + 512],
                        ysb[:, 512 * n:512 * n + 512])

            deferred = [None]  # previous head's normalize(+emit) closure

            for c in range(2):
                for i in range(4):
                    qcol = 1024 * i + 512 * c
                    # steps: (src, t, col0, wsc, mask_spec)
                    steps = [('kb', t, 0, 512, None) for t in range(8)]
                    for (t, col0, mspec, widened) in self_steps[c]:
                        steps.append(('sf', t, col0, 512 - col0, mspec))
                    nst = len(steps)
                    # ops_/pd allocated lazily at the first flush: the "out"
                    # psum ring is shared with emit_y's py tiles, and the
                    # deferred previous-head emit must claim its ring slots
                    # BEFORE this head's accumulator does.
                    acc = {}

                    def get_acc(acc=acc):
                        if 'ops' not in acc:
                            acc['ops'] = psout.tile([128, 512], f32,
                                                    tag="out", name="ops")
                            acc['pd'] = psden.tile([1, 512], f32, tag="den",
                                                   name="pd")
                        return acc['ops'], acc['pd']

                    # group steps into exp units: pairs of 512-wide steps,
                    # singles otherwise
                    units = []
                    j = 0
                    while j < nst:
                        if (j + 1 < nst and steps[j][3] == 512
                                and steps[j + 1][3] == 512):
                            units.append((j, j + 1))
                            j += 2
                        else:
                            units.append((j,))
                            j += 1

                    pending = []  # av/den jobs

                    def flush(jobs, get_acc=get_acc, nst=nst):
                        ops_, pd = get_acc()
                        for (at_sb, atoff, vt_l, c0av, wav, jidx) in jobs:
                            first = (jidx == 0)
                            last = (jidx == nst - 1)
                            nc.tensor.matmul(
                                ops_[:, c0av:c0av + wav], vt_l,
                                at_sb[:, atoff:atoff + wav],
                                start=first, stop=last)
                            nc.tensor.matmul(
                                pd[:, c0av:c0av + wav], ones_sb[:, 0:1],
                                at_sb[:, atoff:atoff + wav],
                                start=first, stop=last)

                    for ui, unit in enumerate(units):
                        sc = pssc.tile([128, 1024], f32, tag="sc", name="sc")
                        at_t = atp.tile([128, 1024], f32r, tag="at",
                                        name="at")
                        jobs = []
                        off = 0
                        for j in unit:
                            src, t, col0, wsc, mspec = steps[j]
                            if src == 'kb':
                                lhsT = kbk_sb[:, 128 * t:128 * t + 128]
                                rhs = qnT[:, qcol:qcol + 512]
                                vt_l = kbv_sb[:, 128 * t:128 * t + 128]
                            else:
                                lhsT = kTr[:, 128 * t:128 * t + 128]
                                rhs = qTr[:, qcol + col0:qcol + 512]
                                vt_l = vkm[:, 128 * t:128 * t + 128]
                            nc.tensor.matmul(sc[:, off:off + wsc], lhsT, rhs,
                                             start=True, stop=True)
                            if mspec is not None and mspec[0] == 'tri':
                                _, toff, wadd = mspec
                                nc.vector.tensor_add(
                                    sc[:, off:off + wadd],
                                    sc[:, off:off + wadd],
                                    tri_sb[:, toff:toff + wadd])
                            elif mspec is not None and mspec[0] == 'packed':
                                _, kidx, wadd = mspec
                                nc.vector.tensor_add(
                                    sc[:, off:off + wadd],
                                    sc[:, off:off + wadd],
                                    mask_sb[:, 512 * kidx:512 * kidx + wadd])
                            # widened tiles: leading 128 cols are masked to
                            # exp()=0; av/den include them (adding zeros)
                            jobs.append((at_t, off, vt_l, col0, wsc, j))
                            off += wsc
                        nc.scalar.activation(at_t[:, 0:off], sc[:, 0:off],
                                             AF.Exp, scale=SCALE)
                        if ui == 0 and deferred[0] is not None:
                            # run previous head's normalize/emit now: its
                            # DVE reciprocal chain overlaps our scores
                            deferred[0]()
                            deferred[0] = None
                        if pending:
                            flush(pending)
                        pending = jobs
                    flush(pending)
                    ops_, pd = get_acc()

                    def normalize(ops_=ops_, pd=pd, qcol=qcol, c=c, i=i):
                        rec32 = nrm.tile([1, 512], f32, tag="rec32")
                        nc.vector.reciprocal_approx_fast(rec32[:], pd[:])
                        rec = nrm.tile([1, 512], f32r, tag="rec")
                        nc.vector.tensor_copy(rec[:], rec32[:])
                        bc = psbc.tile([128, 512], f32, tag="bc")
                        nc.tensor.matmul(bc[:], ones_sb[0:1, :], rec[:],
                                         start=True, stop=True)
                        bc_sb = nrm.tile([128, 512], f32r, tag="bc_sb")
                        nc.vector.tensor_copy(bc_sb[:], bc[:])
                        nc.vector.tensor_mul(outn[:, qcol:qcol + 512],
                                             ops_[:], bc_sb[:])
                        if c == 1:
                            emit_y_tile(i)
                            if i == 2:
                                # heads 0-2 done: emit partial y tiles 4-7
                                for st in range(4, 8):
                                    emit_y_partial(st)

                    deferred[0] = normalize
            deferred[0]()
            for st in range(4, 8):
                emit_y_final(st)

    nc.compile()
    return nc


def kernel(hidden_states, attention_mask, position_ids, kb_keys, kb_values,
           Wq, Wq_new, Wk, Wv, Wo):
    from concourse.bass_utils import run_bass_kernel_spmd

    hidden_states = np.asarray(hidden_states, dtype=np.float32)
    attention_mask = np.asarray(attention_mask, dtype=np.float32)
    position_ids = np.asarray(position_ids)
    kb_keys = np.asarray(kb_keys, dtype=np.float32)
    kb_values = np.asarray(kb_values, dtype=np.float32)
    Wq = np.asarray(Wq, dtype=np.float32)
    Wq_new = np.asarray(Wq_new, dtype=np.float32)
    Wk = np.asarray(Wk, dtype=np.float32)
    Wv = np.asarray(Wv, dtype=np.float32)
    Wo = np.asarray(Wo, dtype=np.float32)

    # ---- host: classify self-attention mask blocks ----
    mask = attention_mask[:, 0]  # (B, S, S) [q, key]
    tri_blk = np.where(
        np.arange(128)[None, :] >= np.arange(128)[:, None], 0.0,
        NEG).astype(np.float32)  # [key, q] triangle

    self_steps = {}
    packed = []  # (b-independent) packed fallback mask blocks, [key, q]
    use_tri = False
    for c in range(2):
        lst = []
        for t in range(8):
            blk = mask[:, 512 * c:512 * c + 512, 128 * t:128 * t + 128]
            if np.all(blk <= -1e8):
                continue
            colmask = np.all(blk <= -1e8, axis=(0, 2))  # (512,) per q-col
            col0 = 0
            while col0 < 512 and colmask[col0]:
                col0 += 1
            col0 = (col0 // 128) * 128
            sub = blk[:, col0:, :]  # (B, w, 128) [q, key]
            if not np.any(sub < 0):
                lst.append((t, col0, None, False))
                continue
            # mixed: is it the canonical causal triangle at window start?
            w = 512 - col0
            exp_pat = np.zeros((w, 128), np.float32)
            exp_pat[:128] = tri_blk.T  # [q, key]
            is_tri = all(np.array_equal(sub[b_], exp_pat) for b_ in range(B))
            if is_tri:
                use_tri = True
                if w == 128:
                    # widen to 256 (f32r <256-col moving runs at 1/4 rate);
                    # extra leading block is fully masked -> exp gives 0
                    lst.append((t, col0 - 128, ('tri', 0, 256), True))
                else:
                    lst.append((t, col0, ('tri', 128, 128), False))
            else:
                packed.append((c, t, col0))
                lst.append((t, col0, ('packed', len(packed) - 1, w), False))
        self_steps[c] = lst
    n_mask = len(packed)

    plan = dict(self_steps=self_steps, n_mask=n_mask, use_tri=use_tri)
    nc = _build_program(plan)

    # ---- host: shared constant prep ----
    inv_freq = 1.0 / (THETA ** (np.arange(0, HD, 2, dtype=np.float32) / HD))
    P = np.zeros((HD, HD), np.float32)
    for d in range(64):
        P[d, d + 64] = -1.0
        P[d + 64, d] = 1.0
    ropePT = np.ascontiguousarray(P.T)
    ident = np.eye(128, dtype=np.float32)
    ones_np = np.ones((128, 128), np.float32)
    tri384 = np.concatenate([np.full((128, 128), NEG, np.float32),
                             tri_blk, np.zeros((128, 128), np.float32)],
                            axis=1)

    def pack_w(wT, ndt):
        # wT (H, 128*ndt) -> (128, 2048*ndt): tile (dt) block holds 16
        # h-tiles side by side: cols 2048*dt + 128*h = wT[128h:+128, 128dt:+128]
        out = np.empty((128, 2048 * ndt), np.float32)
        for dt_i in range(ndt):
            for h in range(16):
                out[:, 2048 * dt_i + 128 * h:2048 * dt_i + 128 * h + 128] = \
                    wT[128 * h:128 * h + 128, 128 * dt_i:128 * dt_i + 128]
        return out

    cosTs, sinTs, maskTs = [], [], []
    for b in range(B):
        freqs = position_ids[b].astype(np.float32)[:, None] * inv_freq[None, :]
        emb = np.concatenate([freqs, freqs], axis=1)  # (S, 128)
        cosTs.append(np.ascontiguousarray(np.cos(emb).T.astype(np.float32)))
        sinTs.append(np.ascontiguousarray(np.sin(emb).T.astype(np.float32)))
        if n_mask:
            mt = np.zeros((128, 512 * n_mask), np.float32)
            for kidx, (c, t, col0) in enumerate(packed):
                w = 512 - col0
                mt[:, 512 * kidx:512 * kidx + w] = \
                    mask[b, 512 * c + col0:512 * c + 512,
                         128 * t:128 * t + 128].T
            maskTs.append(mt)

    in_maps = []
    for cid in range(8):
        b, g = cid // 4, cid % 4
        kbv_p = np.empty((128, KB), np.float32)
        kvb = kb_values[b, :, 128 * g:128 * g + 128]
        for t in range(8):
            kbv_p[:, 128 * t:128 * t + 128] = kvb[128 * t:128 * t + 128, :]
        wo_p = np.empty((128, 8192), np.float32)
        woT = Wo[:, 512 * g:512 * g + 512].T  # (512, 2048)
        for i in range(4):
            wo_p[:, 2048 * i:2048 * i + 2048] = woT[128 * i:128 * i + 128, :]
        m = dict(
            xT=np.ascontiguousarray(hidden_states[b].T),
            wq=pack_w(Wq[512 * g:512 * g + 512, :].T, 4),
            wqn=pack_w(Wq_new[512 * g:512 * g + 512, :].T, 4),
            wk=pack_w(Wk[128 * g:128 * g + 128, :].T, 1),
            wv=pack_w(Wv[128 * g:128 * g + 128, :].T, 1),
            wo=wo_p,
            kbkT=np.ascontiguousarray(kb_keys[b, :, 128 * g:128 * g + 128].T),
            kbv=kbv_p,
            cosT=cosTs[b], sinT=sinTs[b],
            ropePT=ropePT, ident=ident, ones=ones_np,
        )
        if use_tri:
            m['tri'] = tri384
        if n_mask:
            m['masks'] = maskTs[b]
        in_maps.append(m)

    res = run_bass_kernel_spmd(nc, in_maps, core_ids=list(range(8)))
    global LAST_RESULTS
    LAST_RESULTS = res

    out = np.zeros((B, S, H), np.float32)
    for cid in range(8):
        b = cid // 4
        out[b] += res.results[cid]["y"]
    return out


# revision 18
# speedup vs baseline: 1.1408x; 1.0530x over previous
"""Trainium2 Bass kernel for KBLAM Gemma3n attention (B=2, S=1024, H=2048,
NH=16, NKV=4, HD=128, KB=1024), sharded over 8 NeuronCores as
(batch x kv-head-group): core = 4*b + g handles batch b and kv head g
(which serves q-heads 4g..4g+3).  Each core computes a partial s-major
output y_part (S, H) = attn_out @ Wo[:, 512g:512g+512].T ; the host sums
the 4 partials per batch.

v3 design notes (v1 baseline 267us; v2 fp16 experiment showed fp16
ldweights serialize with fp16 matmuls, so the value path stays f32r):
 - fp16 warmup matmuls on memset data ramp the PE p-state during DMA boot.
 - DMA order tuned for time-to-first-matmul: wk split in 2, then the xt
   stream with wq0 pieces interleaved; k/v/q0 projections ride the xt
   stream (interleaved per h-tile, 6 psum banks).
 - rope / v-transpose / psum->sbuf copies pipelined behind the NEXT
   chunk's matmuls so the PE never waits on the DVE.
 - score tiles that would be 128 cols wide are widened to 256 (f32r
   moving <256 cols runs at 1/4 rate; 256 at full rate) with the extra
   block masked via the triangle tile; av/den include the zeroed block.
 - causal masks: one [128, 384] (neg | tri | 0) f32 tile replaces 2.1MB
   of packed per-block masks (generic packed fallback kept).
 - exp processed in [128, 1024] psum pairs (two 512-wide score tiles
   share one activation instruction) to halve Act-engine overhead.
 - per-head normalization (reciprocal chain + y emit) deferred into the
   next head's first attention unit so the PE never waits on the DVE.
 - y tiles 4-7: heads 0-2 partial sums emitted during head 3's
   attention; only the head-3 contribution + add + quadrant DMA remain
   in the tail.
"""
import math
from contextlib import ExitStack

import numpy as np

B, S, H = 2, 1024, 2048
NH, NKV, HD = 16, 4, 128
KB = 1024
THETA = 10000.0
SCALE = 1.0 / math.sqrt(HD)
NEG = -1e9

LAST_RESULTS = None


def _build_program(plan):
    """Build the single-core Bass/Tile program.

    plan: dict with
      self_steps: {c: [(t, col0, mask_spec, widened), ...]} mask_spec is
        ('tri', off, w_add) or ('packed', k, w_add) or None
      n_mask: number of packed [128, 512] fallback mask tiles
      use_tri: whether the triangle tile input is present
    """
    import concourse.tile as tile
    from concourse import bacc, mybir

    f32 = mybir.dt.float32
    f32r = mybir.dt.float32r
    f16 = mybir.dt.float16
    AF = mybir.ActivationFunctionType
    nc = bacc.Bacc("TRN2", target_bir_lowering=False, debug=False,
                   enable_asserts=False, num_devices=8)

    self_steps = plan['self_steps']
    n_mask = plan['n_mask']
    use_tri = plan['use_tri']

    xT = nc.dram_tensor("xT", [H, S], f32r, kind="ExternalInput")
    # packed weights: per-dt blocks of 16 h-tiles: cols 2048*dt + 128*h
    wq = nc.dram_tensor("wq", [128, 8192], f32r, kind="ExternalInput")
    wqn = nc.dram_tensor("wqn", [128, 8192], f32r, kind="ExternalInput")
    wk = nc.dram_tensor("wk", [128, 2048], f32r, kind="ExternalInput")
    wv = nc.dram_tensor("wv", [128, 2048], f32r, kind="ExternalInput")
    # wo packed: block i at cols 2048*i = Wo_g^T[128i:128i+128, :]
    wo = nc.dram_tensor("wo", [128, 8192], f32r, kind="ExternalInput")
    kbkT = nc.dram_tensor("kbkT", [128, KB], f32r, kind="ExternalInput")
    # kbv packed key-major tiles side by side: tile t at cols 128*t
    kbv = nc.dram_tensor("kbv", [128, KB], f32r, kind="ExternalInput")
    cosT = nc.dram_tensor("cosT", [128, S], f32r, kind="ExternalInput")
    sinT = nc.dram_tensor("sinT", [128, S], f32r, kind="ExternalInput")
    ropePT = nc.dram_tensor("ropePT", [128, 128], f32r, kind="ExternalInput")
    ident = nc.dram_tensor("ident", [128, 128], f32r, kind="ExternalInput")
    ones = nc.dram_tensor("ones", [128, 128], f32r, kind="ExternalInput")
    if use_tri:
        tri = nc.dram_tensor("tri", [128, 384], f32, kind="ExternalInput")
    if n_mask:
        masks = nc.dram_tensor("masks", [128, 512 * n_mask], f32,
                               kind="ExternalInput")
    y = nc.dram_tensor("y", [S, H], f32, kind="ExternalOutput")
    # head-3 contribution to y rows 512-1023, summed into y on the HOST:
    # it is the only end-gated data, so it ships in bf16 to halve the
    # bandwidth-bound tail DMA.
    bf16 = mybir.dt.bfloat16
    y3 = nc.dram_tensor("y3", [S // 2, H], bf16, kind="ExternalOutput")

    with tile.TileContext(nc) as tc, ExitStack() as ctx:
        po = ctx.enter_context(tc.tile_pool(name="projout", bufs=1))
        qTr = po.tile([128, 4096], f32r, tag="qTr")
        qnT = po.tile([128, 4096], f32r, tag="qnT")
        kTr = po.tile([128, 1024], f32r, tag="kTr")
        vkm = po.tile([128, 1024], f32r, tag="vkm")
        outn = po.tile([128, 4096], f32r, tag="outn")

        consts = ctx.enter_context(tc.tile_pool(name="consts", bufs=1))
        kbp = ctx.enter_context(tc.tile_pool(name="kb", bufs=1))

        # ---------------- phase 1: projections + rope + v transpose ------
        with tc.tile_pool(name="xw", bufs=1) as xw, \
             tc.tile_pool(name="wt", bufs=5) as wpool, \
             tc.tile_pool(name="ptmp", bufs=3) as ptmp, \
             tc.tile_pool(name="psp", bufs=6, space="PSUM") as psp, \
             tc.tile_pool(name="psr", bufs=2, space="PSUM") as psr:

            # -- warmup: ramp the PE p-state while DMA boots --------------
            warm_sb = xw.tile([128, 512], f16, tag="warm")
            nc.vector.memset(warm_sb[:], 1.0)
            for wi in range(24):
                wps = psp.tile([128, 512], f32, tag="ck", name="wps")
                nc.tensor.matmul(wps[:], warm_sb[:, 0:128], warm_sb[:],
                                 start=True, stop=True)

            # -- DMA emission order = arrival order (FIFO over 16 engines)
            wblk_k = wpool.tile([128, 2048], f32r, tag="wblk", name="wblk_k")
            for p in range(2):
                nc.sync.dma_start(wblk_k[:, 1024 * p:1024 * p + 1024],
                                  wk[:, 1024 * p:1024 * p + 1024])
            wblk_v = wpool.tile([128, 2048], f32r, tag="wblk", name="wblk_v")
            nc.sync.dma_start(wblk_v[:], wv[:])
            xt = xw.tile([128, 16384], f32r, tag="xt")
            wblk_q0 = wpool.tile([128, 2048], f32r, tag="wblk",
                                 name="wblk_q0")
            for h in range(16):
                nc.sync.dma_start(xt[:, 1024 * h:1024 * h + 1024],
                                  xT[128 * h:128 * h + 128, :])
                # wq dt0 halves interleaved into the xt stream
                if h in (3, 5):
                    p = (3, 5).index(h)
                    nc.sync.dma_start(wblk_q0[:, 1024 * p:1024 * p + 1024],
                                      wq[:, 1024 * p:1024 * p + 1024])
            # remaining weights + consts, in need-order
            wblks = {('q', 0): wblk_q0}
            loads = [('qn', 0, wqn), ('q', 1, wq), ('qn', 1, wqn)]
            for kind, dt_i, src in loads:
                t_ = wpool.tile([128, 2048], f32r, tag="wblk", name="wblk")
                nc.sync.dma_start(t_[:], src[:, 2048 * dt_i:2048 * dt_i + 2048])
                wblks[(kind, dt_i)] = t_
            cos_sb = consts.tile([128, S], f32r, tag="cos")
            nc.sync.dma_start(cos_sb[:], cosT[:])
            sin_sb = consts.tile([128, S], f32r, tag="sin")
            nc.sync.dma_start(sin_sb[:], sinT[:])
            rp_sb = consts.tile([128, 128], f32r, tag="rp")
            nc.sync.dma_start(rp_sb[:], ropePT[:])
            id_sb = consts.tile([128, 128], f32r, tag="id")
            nc.sync.dma_start(id_sb[:], ident[:])
            ones_sb = consts.tile([128, 128], f32r, tag="ones")
            nc.sync.dma_start(ones_sb[:], ones[:])
            if use_tri:
                tri_sb = consts.tile([128, 384], f32, tag="tri")
                nc.sync.dma_start(tri_sb[:], tri[:])
            if n_mask:
                mask_sb = consts.tile([128, 512 * n_mask], f32, tag="mask")
                nc.sync.dma_start(mask_sb[:], masks[:])
            for kind, dt_i, src in [('q', 2, wq), ('qn', 2, wqn),
                                    ('q', 3, wq), ('qn', 3, wqn)]:
                t_ = wpool.tile([128, 2048], f32r, tag="wblk", name="wblk")
                nc.sync.dma_start(t_[:], src[:, 2048 * dt_i:2048 * dt_i + 2048])
                wblks[(kind, dt_i)] = t_
            kbk_sb = kbp.tile([128, KB], f32r, tag="kbk")
            nc.sync.dma_start(kbk_sb[:], kbkT[:])
            kbv_sb = kbp.tile([128, KB], f32r, tag="kbv")
            nc.sync.dma_start(kbv_sb[:], kbv[:])

            vt_tmp = xw.tile([128, 1024], f32r, tag="vt")

            def chunk_mms(wblk, pss):
                for h in range(16):
                    for half in range(2):
                        nc.tensor.matmul(
                            pss[half][:], wblk[:, 128 * h:128 * h + 128],
                            xt[:, 1024 * h + 512 * half:
                               1024 * h + 512 * half + 512],
                            start=(h == 0), stop=(h == 15))

            def rope_half(tmp, half, dst):
                # dst = tmp*cos + (P@tmp)*sin  (tmp already in SBUF f32r)
                pp = psr.tile([128, 512], f32, tag="aux", name="pp")
                nc.tensor.matmul(pp[:], rp_sb[:], tmp[:], start=True,
                                 stop=True)
                cs = cos_sb[:, 512 * half:512 * half + 512]
                sn = sin_sb[:, 512 * half:512 * half + 512]
                nc.vector.tensor_mul(dst, tmp[:], cs)
                tmp2 = ptmp.tile([128, 512], f32r, tag="tmp2")
                nc.vector.tensor_mul(tmp2[:], pp[:], sn)
                nc.vector.tensor_add(dst, dst, tmp2[:])

            # --- staggered triple (k, v lag 4, q0 lag 8) on the xt stream.
            # k alone leads (only wk+xt0 needed to start); v/q0 join as
            # their weights arrive, keeping the PE packed while xt streams.
            pk = [psp.tile([128, 512], f32, tag="ck", name="pk")
                  for _ in range(2)]
            pv = [psp.tile([128, 512], f32, tag="ck", name="pv")
                  for _ in range(2)]
            pq0 = [psp.tile([128, 512], f32, tag="ck", name="pq0")
                   for _ in range(2)]

            def proj_mm(pss, wblk, h, first, last):
                for half in range(2):
                    nc.tensor.matmul(
                        pss[half][:], wblk[:, 128 * h:128 * h + 128],
                        xt[:, 1024 * h + 512 * half:1024 * h + 512 * half + 512],
                        start=first, stop=last)

            for s in range(24):
                if s < 16:
                    proj_mm(pk, wblk_k, s, s == 0, s == 15)
                if 4 <= s < 20:
                    proj_mm(pv, wblk_v, s - 4, s == 4, s == 19)
                if 8 <= s < 24:
                    proj_mm(pq0, wblk_q0, s - 8, s == 8, s == 23)

            # psum -> sbuf copies (DVE) free the 6 banks for later chunks
            tmps = {}
            for nm, pp_ in (('k0', pk[0]), ('k1', pk[1]),
                            ('q00', pq0[0]), ('q01', pq0[1])):
                t_ = ptmp.tile([128, 512], f32r, tag="tmp", name="t" + nm,
                               bufs=6)
                nc.vector.tensor_copy(t_[:], pp_[:])
                tmps[nm] = t_
            for half in range(2):
                nc.vector.tensor_copy(
                    vt_tmp[:, 512 * half:512 * half + 512], pv[half][:])

            # v transposes: fill the gap until wqn0 arrives
            for t in range(8):
                pst = psr.tile([128, 128], f32r, tag="aux", name="ptr")
                nc.tensor.transpose(
                    pst[:], vt_tmp[:, 128 * t:128 * t + 128], id_sb[:])
                nc.vector.tensor_copy(vkm[:, 128 * t:128 * t + 128], pst[:])

            # --- remaining chunks, post-processing pipelined one behind --
            post_q = []  # deferred rope/copy closures

            def post_k():
                for half in range(2):
                    rope_half(tmps['k' + str(half)], half,
                              kTr[:, 512 * half:512 * half + 512])

            def mk_post_q(tm0, tm1, dt_i):
                def f():
                    rope_half(tm0, 0, qTr[:, 1024 * dt_i:1024 * dt_i + 512])
                    rope_half(tm1, 1,
                              qTr[:, 1024 * dt_i + 512:1024 * dt_i + 1024])
                return f

            def mk_post_qn(p0, p1, dt_i):
                def f():
                    nc.vector.tensor_copy(
                        qnT[:, 1024 * dt_i:1024 * dt_i + 512], p0[:])
                    nc.scalar.activation(
                        qnT[:, 1024 * dt_i + 512:1024 * dt_i + 1024], p1[:],
                        AF.Copy)
                return f

            post_q.append(post_k)
            post_q.append(mk_post_q(tmps['q00'], tmps['q01'], 0))

            rest = [('qn', 0), ('q', 1), ('qn', 1), ('q', 2), ('qn', 2),
                    ('q', 3), ('qn', 3)]
            for kind, dt_i in rest:
                pss = [psp.tile([128, 512], f32, tag="ck", name="pc0"),
                       psp.tile([128, 512], f32, tag="ck", name="pc1")]
                chunk_mms(wblks[(kind, dt_i)], pss)
                # run one deferred post-processing batch behind these mms
                if post_q:
                    post_q.pop(0)()
                if kind == 'q':
                    tm0 = ptmp.tile([128, 512], f32r, tag="tmp", name="tmq0",
                                    bufs=6)
                    nc.vector.tensor_copy(tm0[:], pss[0][:])
                    tm1 = ptmp.tile([128, 512], f32r, tag="tmp", name="tmq1",
                                    bufs=6)
                    nc.vector.tensor_copy(tm1[:], pss[1][:])
                    post_q.append(mk_post_q(tm0, tm1, dt_i))
                else:
                    post_q.append(mk_post_qn(pss[0], pss[1], dt_i))
            while post_q:
                post_q.pop(0)()

        # ---------------- phase 2: attention ------------------------------
        onp = ctx.enter_context(tc.tile_pool(name="onp", bufs=1))
        wo_sb = onp.tile([128, 8192], f32r, tag="wo")
        nc.sync.dma_start(wo_sb[:], wo[:])

        with tc.tile_pool(name="at", bufs=4) as atp, \
             tc.tile_pool(name="nrm", bufs=3) as nrm, \
             tc.tile_pool(name="pssc", bufs=2, space="PSUM") as pssc, \
             tc.tile_pool(name="psout", bufs=2, space="PSUM") as psout, \
             tc.tile_pool(name="psden", bufs=1, space="PSUM") as psden, \
             tc.tile_pool(name="psbc", bufs=1, space="PSUM") as psbc, \
             tc.tile_pool(name="ysb", bufs=2) as ysbp:

            ncopy = [0]

            def psum_copy(dst, src):
                # alternate DVE / Act for psum->sbuf copies
                ncopy[0] += 1
                if ncopy[0] % 2:
                    nc.vector.tensor_copy(dst, src)
                else:
                    nc.scalar.activation(dst, src, AF.Copy)

            def emit_y_tile(st):
                # full 4-head y tile (used for st 0-3, inline during c=1)
                cy, off = st // 4, 128 * (st % 4)
                ysb = ysbp.tile([128, 2048], f32, tag="ysb", name="ysb",
                                bufs=3)
                for n in range(4):
                    py = psout.tile([128, 512], f32, tag="out", name="py")
                    for i in range(4):
                        lcol = 1024 * i + 512 * cy + off
                        nc.tensor.matmul(
                            py[:], outn[:, lcol:lcol + 128],
                            wo_sb[:, 2048 * i + 512 * n:
                                  2048 * i + 512 * n + 512],
                            start=(i == 0), stop=(i == 3))
                    psum_copy(ysb[:, 512 * n:512 * n + 512], py[:])
                nc.sync.dma_start(y[128 * st:128 * st + 128, :], ysb[:])

            def emit_y_partial(st):
                # heads 0-2 partial for y tile st (st 4-7), DMA'd to y
                # immediately (host adds the y3 head-3 contribution)
                off = 128 * (st % 4)
                ysb = ysbp.tile([128, 2048], f32, tag="ysb", name="ysbp",
                                bufs=3)
                for n in range(4):
                    py = psout.tile([128, 512], f32, tag="out", name="pyp")
                    for i in range(3):
                        lcol = 1024 * i + 512 + off
                        nc.tensor.matmul(
                            py[:], outn[:, lcol:lcol + 128],
                            wo_sb[:, 2048 * i + 512 * n:
                                  2048 * i + 512 * n + 512],
                            start=(i == 0), stop=(i == 2))
                    psum_copy(ysb[:, 512 * n:512 * n + 512], py[:])
                nc.sync.dma_start(y[128 * st:128 * st + 128, :], ysb[:])

            def emit_y_final(st):
                # head-3 contribution -> y3 (bf16), host adds into y
                off = 128 * (st % 4)
                lcol = 1024 * 3 + 512 + off
                y3sb = ysbp.tile([128, 2048], bf16, tag="y3sb", name="y3sb",
                                 bufs=2)
                for n in range(4):
                    py = psout.tile([128, 512], f32, tag="out", name="pyf")
                    nc.tensor.matmul(
                        py[:], outn[:, lcol:lcol + 128],
                        wo_sb[:, 2048 * 3 + 512 * n:2048 * 3 + 512 * n + 512],
                        start=True, stop=True)
                    psum_copy(y3sb[:, 512 * n:512 * n + 512], py[:])
                nc.sync.dma_start(y3[128 * (st - 4):128 * (st - 4) + 128, :],
                                  y3sb[:])

            deferred = [None]  # previous head's normalize(+emit) closure

            for c in range(2):
                for i in range(4):
                    qcol = 1024 * i + 512 * c
                    # steps: (src, t, col0, wsc, mask_spec)
                    steps = [('kb', t, 0, 512, None) for t in range(8)]
                    for (t, col0, mspec, widened) in self_steps[c]:
                        steps.append(('sf', t, col0, 512 - col0, mspec))
                    nst = len(steps)
                    # ops_/pd allocated lazily at the first flush: the "out"
                    # psum ring is shared with emit_y's py tiles, and the
                    # deferred previous-head emit must claim its ring slots
                    # BEFORE this head's accumulator does.
                    acc = {}

                    def get_acc(acc=acc):
                        if 'ops' not in acc:
                            acc['ops'] = psout.tile([128, 512], f32,
                                                    tag="out", name="ops")
                            acc['pd'] = psden.tile([1, 512], f32, tag="den",
                                                   name="pd")
                        return acc['ops'], acc['pd']

                    # group steps into exp units: pairs of 512-wide steps,
                    # singles otherwise
                    units = []
                    j = 0
                    while j < nst:
                        if (j + 1 < nst and steps[j][3] == 512
                                and steps[j + 1][3] == 512):
                            units.append((j, j + 1))
                            j += 2
                        else:
                            units.append((j,))
                            j += 1

                    pending = []  # av/den jobs

                    def flush(jobs, get_acc=get_acc, nst=nst):
                        ops_, pd = get_acc()
                        for (at_sb, atoff, vt_l, c0av, wav, jidx) in jobs:
                            first = (jidx == 0)
                            last = (jidx == nst - 1)
                            nc.tensor.matmul(
                                ops_[:, c0av:c0av + wav], vt_l,
                                at_sb[:, atoff:atoff + wav],
                                start=first, stop=last)
                            nc.tensor.matmul(
                                pd[:, c0av:c0av + wav], ones_sb[:, 0:1],
                                at_sb[:, atoff:atoff + wav],
                                start=first, stop=last)

                    for ui, unit in enumerate(units):
                        sc = pssc.tile([128, 1024], f32, tag="sc", name="sc")
                        at_t = atp.tile([128, 1024], f32r, tag="at",
                                        name="at")
                        jobs = []
                        off = 0
                        for j in unit:
                            src, t, col0, wsc, mspec = steps[j]
                            if src == 'kb':
                                lhsT = kbk_sb[:, 128 * t:128 * t + 128]
                                rhs = qnT[:, qcol:qcol + 512]
                                vt_l = kbv_sb[:, 128 * t:128 * t + 128]
                            else:
                                lhsT = kTr[:, 128 * t:128 * t + 128]
                                rhs = qTr[:, qcol + col0:qcol + 512]
                                vt_l = vkm[:, 128 * t:128 * t + 128]
                            nc.tensor.matmul(sc[:, off:off + wsc], lhsT, rhs,
                                             start=True, stop=True)
                            if mspec is not None and mspec[0] == 'tri':
                                _, toff, wadd = mspec
                                nc.vector.tensor_add(
                                    sc[:, off:off + wadd],
                                    sc[:, off:off + wadd],
                                    tri_sb[:, toff:toff + wadd])
                            elif mspec is not None and mspec[0] == 'packed':
                                _, kidx, wadd = mspec
                                nc.vector.tensor_add(
                                    sc[:, off:off + wadd],
                                    sc[:, off:off + wadd],
                                    mask_sb[:, 512 * kidx:512 * kidx + wadd])
                            # widened tiles: leading 128 cols are masked to
                            # exp()=0; av/den include them (adding zeros)
                            jobs.append((at_t, off, vt_l, col0, wsc, j))
                            off += wsc
                        nc.scalar.activation(at_t[:, 0:off], sc[:, 0:off],
                                             AF.Exp, scale=SCALE)
                        if ui == 0 and deferred[0] is not None:
                            # run previous head's normalize/emit now: its
                            # DVE reciprocal chain overlaps our scores
                            deferred[0]()
                            deferred[0] = None
                        # flush with a lag of TWO exp units so the av/den
                        # matmuls never wait on the scores->mask->exp chain
                        if len(pending) >= 2:
                            flush(pending.pop(0))
                        pending.append(jobs)
                    for jb in pending:
                        flush(jb)
                    ops_, pd = get_acc()

                    def normalize(ops_=ops_, pd=pd, qcol=qcol, c=c, i=i):
                        rec32 = nrm.tile([1, 512], f32, tag="rec32")
                        nc.vector.reciprocal_approx_fast(rec32[:], pd[:])
                        rec = nrm.tile([1, 512], f32r, tag="rec")
                        nc.vector.tensor_copy(rec[:], rec32[:])
                        bc = psbc.tile([128, 512], f32, tag="bc")
                        nc.tensor.matmul(bc[:], ones_sb[0:1, :], rec[:],
                                         start=True, stop=True)
                        bc_sb = nrm.tile([128, 512], f32r, tag="bc_sb")
                        nc.vector.tensor_copy(bc_sb[:], bc[:])
                        nc.vector.tensor_mul(outn[:, qcol:qcol + 512],
                                             ops_[:], bc_sb[:])
                        if c == 1:
                            emit_y_tile(i)
                            if i == 2:
                                # heads 0-2 done: emit partial y tiles 4-7
                                for st in range(4, 8):
                                    emit_y_partial(st)

                    deferred[0] = normalize
            deferred[0]()
            for st in range(4, 8):
                emit_y_final(st)

    nc.compile()
    return nc


def kernel(hidden_states, attention_mask, position_ids, kb_keys, kb_values,
           Wq, Wq_new, Wk, Wv, Wo):
    from concourse.bass_utils import run_bass_kernel_spmd

    hidden_states = np.asarray(hidden_states, dtype=np.float32)
    attention_mask = np.asarray(attention_mask, dtype=np.float32)
    position_ids = np.asarray(position_ids)
    kb_keys = np.asarray(kb_keys, dtype=np.float32)
    kb_values = np.asarray(kb_values, dtype=np.float32)
    Wq = np.asarray(Wq, dtype=np.float32)
    Wq_new = np.asarray(Wq_new, dtype=np.float32)
    Wk = np.asarray(Wk, dtype=np.float32)
    Wv = np.asarray(Wv, dtype=np.float32)
    Wo = np.asarray(Wo, dtype=np.float32)

    # ---- host: classify self-attention mask blocks ----
    mask = attention_mask[:, 0]  # (B, S, S) [q, key]
    tri_blk = np.where(
        np.arange(128)[None, :] >= np.arange(128)[:, None], 0.0,
        NEG).astype(np.float32)  # [key, q] triangle

    self_steps = {}
    packed = []  # (b-independent) packed fallback mask blocks, [key, q]
    use_tri = False
    for c in range(2):
        lst = []
        for t in range(8):
            blk = mask[:, 512 * c:512 * c + 512, 128 * t:128 * t + 128]
            if np.all(blk <= -1e8):
                continue
            colmask = np.all(blk <= -1e8, axis=(0, 2))  # (512,) per q-col
            col0 = 0
            while col0 < 512 and colmask[col0]:
                col0 += 1
            col0 = (col0 // 128) * 128
            sub = blk[:, col0:, :]  # (B, w, 128) [q, key]
            if not np.any(sub < 0):
                lst.append((t, col0, None, False))
                continue
            # mixed: is it the canonical causal triangle at window start?
            w = 512 - col0
            exp_pat = np.zeros((w, 128), np.float32)
            exp_pat[:128] = tri_blk.T  # [q, key]
            is_tri = all(np.array_equal(sub[b_], exp_pat) for b_ in range(B))
            if is_tri:
                use_tri = True
                if w == 128:
                    # widen to 256 (f32r <256-col moving runs at 1/4 rate);
                    # extra leading block is fully masked -> exp gives 0
                    lst.append((t, col0 - 128, ('tri', 0, 256), True))
                else:
                    lst.append((t, col0, ('tri', 128, 128), False))
            else:
                packed.append((c, t, col0))
                lst.append((t, col0, ('packed', len(packed) - 1, w), False))
        self_steps[c] = lst
    n_mask = len(packed)

    plan = dict(self_steps=self_steps, n_mask=n_mask, use_tri=use_tri)
    nc = _build_program(plan)

    # ---- host: shared constant prep ----
    inv_freq = 1.0 / (THETA ** (np.arange(0, HD, 2, dtype=np.float32) / HD))
    P = np.zeros((HD, HD), np.float32)
    for d in range(64):
        P[d, d + 64] = -1.0
        P[d + 64, d] = 1.0
    ropePT = np.ascontiguousarray(P.T)
    ident = np.eye(128, dtype=np.float32)
    ones_np = np.ones((128, 128), np.float32)
    tri384 = np.concatenate([np.full((128, 128), NEG, np.float32),
                             tri_blk, np.zeros((128, 128), np.float32)],
                            axis=1)

    def pack_w(wT, ndt):
        # wT (H, 128*ndt) -> (128, 2048*ndt): tile (dt) block holds 16
        # h-tiles side by side: cols 2048*dt + 128*h = wT[128h:+128, 128dt:+128]
        out = np.empty((128, 2048 * ndt), np.float32)
        for dt_i in range(ndt):
            for h in range(16):
                out[:, 2048 * dt_i + 128 * h:2048 * dt_i + 128 * h + 128] = \
                    wT[128 * h:128 * h + 128, 128 * dt_i:128 * dt_i + 128]
        return out

    cosTs, sinTs, maskTs = [], [], []
    for b in range(B):
        freqs = position_ids[b].astype(np.float32)[:, None] * inv_freq[None, :]
        emb = np.concatenate([freqs, freqs], axis=1)  # (S, 128)
        cosTs.append(np.ascontiguousarray(np.cos(emb).T.astype(np.float32)))
        sinTs.append(np.ascontiguousarray(np.sin(emb).T.astype(np.float32)))
        if n_mask:
            mt = np.zeros((128, 512 * n_mask), np.float32)
            for kidx, (c, t, col0) in enumerate(packed):
                w = 512 - col0
                mt[:, 512 * kidx:512 * kidx + w] = \
                    mask[b, 512 * c + col0:512 * c + 512,
                         128 * t:128 * t + 128].T
            maskTs.append(mt)

    in_maps = []
    for cid in range(8):
        b, g = cid // 4, cid % 4
        kbv_p = np.empty((128, KB), np.float32)
        kvb = kb_values[b, :, 128 * g:128 * g + 128]
        for t in range(8):
            kbv_p[:, 128 * t:128 * t + 128] = kvb[128 * t:128 * t + 128, :]
        wo_p = np.empty((128, 8192), np.float32)
        woT = Wo[:, 512 * g:512 * g + 512].T  # (512, 2048)
        for i in range(4):
            wo_p[:, 2048 * i:2048 * i + 2048] = woT[128 * i:128 * i + 128, :]
        m = dict(
            xT=np.ascontiguousarray(hidden_states[b].T),
            wq=pack_w(Wq[512 * g:512 * g + 512, :].T, 4),
            wqn=pack_w(Wq_new[512 * g:512 * g + 512, :].T, 4),
            wk=pack_w(Wk[128 * g:128 * g + 128, :].T, 1),
            wv=pack_w(Wv[128 * g:128 * g + 128, :].T, 1),
            wo=wo_p,
            kbkT=np.ascontiguousarray(kb_keys[b, :, 128 * g:128 * g + 128].T),
            kbv=kbv_p,
            cosT=cosTs[b], sinT=sinTs[b],
            ropePT=ropePT, ident=ident, ones=ones_np,
        )
        if use_tri:
            m['tri'] = tri384
        if n_mask:
            m['masks'] = maskTs[b]
        in_maps.append(m)

    res = run_bass_kernel_spmd(nc, in_maps, core_ids=list(range(8)))
    global LAST_RESULTS
    LAST_RESULTS = res

    out = np.zeros((B, S, H), np.float32)
    for cid in range(8):
        b = cid // 4
        out[b] += res.results[cid]["y"]
        out[b, S // 2:] += res.results[cid]["y3"].astype(np.float32)
    return out


# revision 23
# speedup vs baseline: 1.1536x; 1.0112x over previous
"""Trainium2 Bass kernel for KBLAM Gemma3n attention (B=2, S=1024, H=2048,
NH=16, NKV=4, HD=128, KB=1024), sharded over 8 NeuronCores as
(batch x kv-head-group): core = 4*b + g handles batch b and kv head g
(which serves q-heads 4g..4g+3).  Each core computes a partial s-major
output y_part (S, H) = attn_out @ Wo[:, 512g:512g+512].T ; the host sums
the 4 partials per batch.

v3 design notes (v1 baseline 267us; v2 fp16 experiment showed fp16
ldweights serialize with fp16 matmuls, so the value path stays f32r):
 - fp16 warmup matmuls on memset data ramp the PE p-state during DMA boot.
 - DMA order tuned for time-to-first-matmul: wk split in 2, then the xt
   stream with wq0 pieces interleaved; k/v/q0 projections ride the xt
   stream (interleaved per h-tile, 6 psum banks).
 - rope / v-transpose / psum->sbuf copies pipelined behind the NEXT
   chunk's matmuls so the PE never waits on the DVE.
 - score tiles that would be 128 cols wide are widened to 256 (f32r
   moving <256 cols runs at 1/4 rate; 256 at full rate) with the extra
   block masked via the triangle tile; av/den include the zeroed block.
 - causal masks: one [128, 384] (neg | tri | 0) f32 tile replaces 2.1MB
   of packed per-block masks (generic packed fallback kept).
 - exp processed in [128, 1024] psum pairs (two 512-wide score tiles
   share one activation instruction) to halve Act-engine overhead.
 - per-head normalization (reciprocal chain + y emit) deferred into the
   next head's first attention unit so the PE never waits on the DVE.
 - y tiles 4-7: heads 0-2 partial sums emitted during head 3's
   attention; only the head-3 contribution + add + quadrant DMA remain
   in the tail.
"""
import math
from contextlib import ExitStack

import numpy as np

B, S, H = 2, 1024, 2048
NH, NKV, HD = 16, 4, 128
KB = 1024
THETA = 10000.0
SCALE = 1.0 / math.sqrt(HD)
NEG = -1e9

LAST_RESULTS = None


def _build_program(plan):
    """Build the single-core Bass/Tile program.

    plan: dict with
      self_steps: {c: [(t, col0, mask_spec, widened), ...]} mask_spec is
        ('tri', off, w_add) or ('packed', k, w_add) or None
      n_mask: number of packed [128, 512] fallback mask tiles
      use_tri: whether the triangle tile input is present
    """
    import concourse.tile as tile
    from concourse import bacc, mybir

    f32 = mybir.dt.float32
    f32r = mybir.dt.float32r
    f16 = mybir.dt.float16
    AF = mybir.ActivationFunctionType
    nc = bacc.Bacc("TRN2", target_bir_lowering=False, debug=False,
                   enable_asserts=False, num_devices=8)

    self_steps = plan['self_steps']
    n_mask = plan['n_mask']
    use_tri = plan['use_tri']

    xT = nc.dram_tensor("xT", [H, S], f32r, kind="ExternalInput")
    # packed weights: per-dt blocks of 16 h-tiles: cols 2048*dt + 128*h
    wq = nc.dram_tensor("wq", [128, 8192], f32r, kind="ExternalInput")
    wqn = nc.dram_tensor("wqn", [128, 8192], f32r, kind="ExternalInput")
    wk = nc.dram_tensor("wk", [128, 2048], f32r, kind="ExternalInput")
    wv = nc.dram_tensor("wv", [128, 2048], f32r, kind="ExternalInput")
    # wo packed: block i at cols 2048*i = Wo_g^T[128i:128i+128, :]
    wo = nc.dram_tensor("wo", [128, 8192], f32r, kind="ExternalInput")
    kbkT = nc.dram_tensor("kbkT", [128, KB], f32r, kind="ExternalInput")
    # kbv packed key-major tiles side by side: tile t at cols 128*t
    kbv = nc.dram_tensor("kbv", [128, KB], f32r, kind="ExternalInput")
    cosT = nc.dram_tensor("cosT", [128, S], f32r, kind="ExternalInput")
    sinT = nc.dram_tensor("sinT", [128, S], f32r, kind="ExternalInput")
    ropePT = nc.dram_tensor("ropePT", [128, 128], f32r, kind="ExternalInput")
    ident = nc.dram_tensor("ident", [128, 128], f32r, kind="ExternalInput")
    ones = nc.dram_tensor("ones", [128, 128], f32r, kind="ExternalInput")
    if use_tri:
        tri = nc.dram_tensor("tri", [128, 384], f32, kind="ExternalInput")
    if n_mask:
        masks = nc.dram_tensor("masks", [128, 512 * n_mask], f32,
                               kind="ExternalInput")
    y = nc.dram_tensor("y", [S, H], f32, kind="ExternalOutput")
    # head-3 contribution to y rows 512-1023, summed into y on the HOST:
    # it is the only end-gated data, so it ships in bf16 to halve the
    # bandwidth-bound tail DMA.
    bf16 = mybir.dt.bfloat16
    y3 = nc.dram_tensor("y3", [S // 2, H], bf16, kind="ExternalOutput")

    with tile.TileContext(nc) as tc, ExitStack() as ctx:
        po = ctx.enter_context(tc.tile_pool(name="projout", bufs=1))
        qTr = po.tile([128, 4096], f32r, tag="qTr")
        qnT = po.tile([128, 4096], f32r, tag="qnT")
        kTr = po.tile([128, 1024], f32r, tag="kTr")
        vkm = po.tile([128, 1024], f32r, tag="vkm")
        outn = po.tile([128, 4096], f32r, tag="outn")

        consts = ctx.enter_context(tc.tile_pool(name="consts", bufs=1))
        kbp = ctx.enter_context(tc.tile_pool(name="kb", bufs=1))

        # ---------------- phase 1: projections + rope + v transpose ------
        with tc.tile_pool(name="xw", bufs=1) as xw, \
             tc.tile_pool(name="wt", bufs=5) as wpool, \
             tc.tile_pool(name="ptmp", bufs=3) as ptmp, \
             tc.tile_pool(name="psp", bufs=6, space="PSUM") as psp, \
             tc.tile_pool(name="psr", bufs=2, space="PSUM") as psr:

            # -- warmup: ramp the PE p-state while DMA boots --------------
            warm_sb = xw.tile([128, 512], f16, tag="warm")
            nc.vector.memset(warm_sb[:], 1.0)
            for wi in range(28):
                wps = psp.tile([128, 512], f32, tag="ck", name="wps")
                nc.tensor.matmul(wps[:], warm_sb[:, 0:128], warm_sb[:],
                                 start=True, stop=True)

            # -- DMA emission order = arrival order (FIFO over 16 engines)
            wblk_k = wpool.tile([128, 2048], f32r, tag="wblk", name="wblk_k")
            for p in range(2):
                nc.sync.dma_start(wblk_k[:, 1024 * p:1024 * p + 1024],
                                  wk[:, 1024 * p:1024 * p + 1024])
            wblk_v = wpool.tile([128, 2048], f32r, tag="wblk", name="wblk_v")
            nc.sync.dma_start(wblk_v[:], wv[:])
            xt = xw.tile([128, 16384], f32r, tag="xt")
            wblk_q0 = wpool.tile([128, 2048], f32r, tag="wblk",
                                 name="wblk_q0")
            for h in range(16):
                nc.sync.dma_start(xt[:, 1024 * h:1024 * h + 1024],
                                  xT[128 * h:128 * h + 128, :])
                # wq dt0 halves interleaved into the xt stream
                if h in (3, 5):
                    p = (3, 5).index(h)
                    nc.sync.dma_start(wblk_q0[:, 1024 * p:1024 * p + 1024],
                                      wq[:, 1024 * p:1024 * p + 1024])
            # remaining weights + consts, in need-order
            wblks = {('q', 0): wblk_q0}
            loads = [('qn', 0, wqn), ('q', 1, wq), ('qn', 1, wqn)]
            for kind, dt_i, src in loads:
                t_ = wpool.tile([128, 2048], f32r, tag="wblk", name="wblk")
                nc.sync.dma_start(t_[:], src[:, 2048 * dt_i:2048 * dt_i + 2048])
                wblks[(kind, dt_i)] = t_
            cos_sb = consts.tile([128, S], f32r, tag="cos")
            nc.sync.dma_start(cos_sb[:], cosT[:])
            sin_sb = consts.tile([128, S], f32r, tag="sin")
            nc.sync.dma_start(sin_sb[:], sinT[:])
            rp_sb = consts.tile([128, 128], f32r, tag="rp")
            nc.sync.dma_start(rp_sb[:], ropePT[:])
            id_sb = consts.tile([128, 128], f32r, tag="id")
            nc.sync.dma_start(id_sb[:], ident[:])
            ones_sb = consts.tile([128, 128], f32r, tag="ones")
            nc.sync.dma_start(ones_sb[:], ones[:])
            if use_tri:
                tri_sb = consts.tile([128, 384], f32, tag="tri")
                nc.sync.dma_start(tri_sb[:], tri[:])
            if n_mask:
                mask_sb = consts.tile([128, 512 * n_mask], f32, tag="mask")
                nc.sync.dma_start(mask_sb[:], masks[:])
            for kind, dt_i, src in [('q', 2, wq), ('qn', 2, wqn),
                                    ('q', 3, wq), ('qn', 3, wqn)]:
                t_ = wpool.tile([128, 2048], f32r, tag="wblk", name="wblk")
                nc.sync.dma_start(t_[:], src[:, 2048 * dt_i:2048 * dt_i + 2048])
                wblks[(kind, dt_i)] = t_
            kbk_sb = kbp.tile([128, KB], f32r, tag="kbk")
            nc.sync.dma_start(kbk_sb[:], kbkT[:])
            kbv_sb = kbp.tile([128, KB], f32r, tag="kbv")
            nc.sync.dma_start(kbv_sb[:], kbv[:])

            vt_tmp = xw.tile([128, 1024], f32r, tag="vt")

            def chunk_mms(wblk, pss):
                for h in range(16):
                    for half in range(2):
                        nc.tensor.matmul(
                            pss[half][:], wblk[:, 128 * h:128 * h + 128],
                            xt[:, 1024 * h + 512 * half:
                               1024 * h + 512 * half + 512],
                            start=(h == 0), stop=(h == 15))

            def rope_half(tmp, half, dst):
                # dst = tmp*cos + (P@tmp)*sin  (tmp already in SBUF f32r)
                pp = psr.tile([128, 512], f32, tag="aux", name="pp")
                nc.tensor.matmul(pp[:], rp_sb[:], tmp[:], start=True,
                                 stop=True)
                cs = cos_sb[:, 512 * half:512 * half + 512]
                sn = sin_sb[:, 512 * half:512 * half + 512]
                nc.vector.tensor_mul(dst, tmp[:], cs)
                tmp2 = ptmp.tile([128, 512], f32r, tag="tmp2")
                nc.vector.tensor_mul(tmp2[:], pp[:], sn)
                nc.vector.tensor_add(dst, dst, tmp2[:])

            # --- staggered triple (k, v lag 4, q0 lag 8) on the xt stream.
            # k alone leads (only wk+xt0 needed to start); v/q0 join as
            # their weights arrive, keeping the PE packed while xt streams.
            pk = [psp.tile([128, 512], f32, tag="ck", name="pk")
                  for _ in range(2)]
            pv = [psp.tile([128, 512], f32, tag="ck", name="pv")
                  for _ in range(2)]
            pq0 = [psp.tile([128, 512], f32, tag="ck", name="pq0")
                   for _ in range(2)]

            def proj_mm(pss, wblk, h, first, last):
                for half in range(2):
                    nc.tensor.matmul(
                        pss[half][:], wblk[:, 128 * h:128 * h + 128],
                        xt[:, 1024 * h + 512 * half:1024 * h + 512 * half + 512],
                        start=first, stop=last)

            for s in range(24):
                if s < 16:
                    proj_mm(pk, wblk_k, s, s == 0, s == 15)
                if 4 <= s < 20:
                    proj_mm(pv, wblk_v, s - 4, s == 4, s == 19)
                if 8 <= s < 24:
                    proj_mm(pq0, wblk_q0, s - 8, s == 8, s == 23)

            # psum -> sbuf copies (DVE) free the 6 banks for later chunks
            tmps = {}
            for nm, pp_ in (('k0', pk[0]), ('k1', pk[1]),
                            ('q00', pq0[0]), ('q01', pq0[1])):
                t_ = ptmp.tile([128, 512], f32r, tag="tmp", name="t" + nm,
                               bufs=6)
                nc.vector.tensor_copy(t_[:], pp_[:])
                tmps[nm] = t_
            for half in range(2):
                nc.vector.tensor_copy(
                    vt_tmp[:, 512 * half:512 * half + 512], pv[half][:])

            # v transposes: fill the gap until wqn0 arrives
            for t in range(8):
                pst = psr.tile([128, 128], f32r, tag="aux", name="ptr")
                nc.tensor.transpose(
                    pst[:], vt_tmp[:, 128 * t:128 * t + 128], id_sb[:])
                nc.vector.tensor_copy(vkm[:, 128 * t:128 * t + 128], pst[:])

            # --- remaining chunks, post-processing pipelined one behind --
            post_q = []  # deferred rope/copy closures

            def post_k():
                for half in range(2):
                    rope_half(tmps['k' + str(half)], half,
                              kTr[:, 512 * half:512 * half + 512])

            def mk_post_q(tm0, tm1, dt_i):
                def f():
                    rope_half(tm0, 0, qTr[:, 1024 * dt_i:1024 * dt_i + 512])
                    rope_half(tm1, 1,
                              qTr[:, 1024 * dt_i + 512:1024 * dt_i + 1024])
                return f

            def mk_post_qn(p0, p1, dt_i):
                def f():
                    nc.vector.tensor_copy(
                        qnT[:, 1024 * dt_i:1024 * dt_i + 512], p0[:])
                    nc.scalar.activation(
                        qnT[:, 1024 * dt_i + 512:1024 * dt_i + 1024], p1[:],
                        AF.Copy)
                return f

            post_q.append(post_k)
            post_q.append(mk_post_q(tmps['q00'], tmps['q01'], 0))

            rest = [('qn', 0), ('q', 1), ('qn', 1), ('q', 2), ('qn', 2),
                    ('q', 3), ('qn', 3)]
            for kind, dt_i in rest:
                pss = [psp.tile([128, 512], f32, tag="ck", name="pc0"),
                       psp.tile([128, 512], f32, tag="ck", name="pc1")]
                chunk_mms(wblks[(kind, dt_i)], pss)
                # run one deferred post-processing batch behind these mms
                if post_q:
                    post_q.pop(0)()
                if kind == 'q':
                    tm0 = ptmp.tile([128, 512], f32r, tag="tmp", name="tmq0",
                                    bufs=6)
                    nc.vector.tensor_copy(tm0[:], pss[0][:])
                    tm1 = ptmp.tile([128, 512], f32r, tag="tmp", name="tmq1",
                                    bufs=6)
                    nc.vector.tensor_copy(tm1[:], pss[1][:])
                    post_q.append(mk_post_q(tm0, tm1, dt_i))
                else:
                    post_q.append(mk_post_qn(pss[0], pss[1], dt_i))
            while post_q:
                post_q.pop(0)()

        # ---------------- phase 2: attention ------------------------------
        onp = ctx.enter_context(tc.tile_pool(name="onp", bufs=1))
        wo_sb = onp.tile([128, 8192], f32r, tag="wo")
        nc.sync.dma_start(wo_sb[:], wo[:])

        with tc.tile_pool(name="at", bufs=4) as atp, \
             tc.tile_pool(name="nrm", bufs=3) as nrm, \
             tc.tile_pool(name="pssc", bufs=2, space="PSUM") as pssc, \
             tc.tile_pool(name="psout", bufs=3, space="PSUM") as psout, \
             tc.tile_pool(name="psden", bufs=1, space="PSUM") as psden, \
             tc.tile_pool(name="ysb", bufs=2) as ysbp:

            ncopy = [0]

            def psum_copy(dst, src):
                # alternate DVE / Act for psum->sbuf copies
                ncopy[0] += 1
                if ncopy[0] % 2:
                    nc.vector.tensor_copy(dst, src)
                else:
                    nc.scalar.activation(dst, src, AF.Copy)

            def emit_y_tile(st):
                # full 4-head y tile (used for st 0-3, inline during c=1)
                cy, off = st // 4, 128 * (st % 4)
                ysb = ysbp.tile([128, 2048], f32, tag="ysb", name="ysb",
                                bufs=3)
                for n in range(4):
                    py = psout.tile([128, 512], f32, tag="out", name="py")
                    for i in range(4):
                        lcol = 1024 * i + 512 * cy + off
                        nc.tensor.matmul(
                            py[:], outn[:, lcol:lcol + 128],
                            wo_sb[:, 2048 * i + 512 * n:
                                  2048 * i + 512 * n + 512],
                            start=(i == 0), stop=(i == 3))
                    psum_copy(ysb[:, 512 * n:512 * n + 512], py[:])
                nc.sync.dma_start(y[128 * st:128 * st + 128, :], ysb[:])

            def emit_y_partial(st):
                # heads 0-2 partial for y tile st (st 4-7), DMA'd to y
                # immediately (host adds the y3 head-3 contribution)
                off = 128 * (st % 4)
                ysb = ysbp.tile([128, 2048], f32, tag="ysb", name="ysbp",
                                bufs=3)
                for n in range(4):
                    py = psout.tile([128, 512], f32, tag="out", name="pyp")
                    for i in range(3):
                        lcol = 1024 * i + 512 + off
                        nc.tensor.matmul(
                            py[:], outn[:, lcol:lcol + 128],
                            wo_sb[:, 2048 * i + 512 * n:
                                  2048 * i + 512 * n + 512],
                            start=(i == 0), stop=(i == 2))
                    psum_copy(ysb[:, 512 * n:512 * n + 512], py[:])
                nc.sync.dma_start(y[128 * st:128 * st + 128, :], ysb[:])

            def emit_y_final(st):
                # head-3 contribution -> y3 (bf16), host adds into y
                off = 128 * (st % 4)
                lcol = 1024 * 3 + 512 + off
                y3sb = ysbp.tile([128, 2048], bf16, tag="y3sb", name="y3sb",
                                 bufs=2)
                for n in range(4):
                    py = psout.tile([128, 512], f32, tag="out", name="pyf")
                    nc.tensor.matmul(
                        py[:], outn[:, lcol:lcol + 128],
                        wo_sb[:, 2048 * 3 + 512 * n:2048 * 3 + 512 * n + 512],
                        start=True, stop=True)
                    psum_copy(y3sb[:, 512 * n:512 * n + 512], py[:])
                nc.sync.dma_start(y3[128 * (st - 4):128 * (st - 4) + 128, :],
                                  y3sb[:])

            # two-stage deferral of the previous head's normalization:
            # stage A (DVE reciprocal chain) at ui==0, stage B (bc matmul,
            # outn multiply, y emits) at ui==1 — by then the DVE chain is
            # done and the PE never waits on it.
            defA = [None]
            defB = [None]

            for c in range(2):
                for i in range(4):
                    qcol = 1024 * i + 512 * c
                    # steps: (src, t, col0, wsc, mask_spec)
                    steps = [('kb', t, 0, 512, None) for t in range(8)]
                    for (t, col0, mspec, widened) in self_steps[c]:
                        steps.append(('sf', t, col0, 512 - col0, mspec))
                    nst = len(steps)
                    # ops_/pd allocated lazily at the first flush: the "out"
                    # psum ring is shared with emit_y's py tiles, and the
                    # deferred previous-head emit must claim its ring slots
                    # BEFORE this head's accumulator does.
                    acc = {}

                    def get_acc(acc=acc):
                        if 'ops' not in acc:
                            acc['ops'] = psout.tile([128, 512], f32,
                                                    tag="out", name="ops")
                            acc['pd'] = psden.tile([1, 512], f32, tag="den",
                                                   name="pd")
                        return acc['ops'], acc['pd']

                    # group steps into exp units: pairs of 512-wide steps,
                    # singles otherwise
                    units = []
                    j = 0
                    while j < nst:
                        if (j + 1 < nst and steps[j][3] == 512
                                and steps[j + 1][3] == 512):
                            units.append((j, j + 1))
                            j += 2
                        else:
                            units.append((j,))
                            j += 1

                    pending = []  # av/den jobs

                    def flush(jobs, get_acc=get_acc, nst=nst):
                        ops_, pd = get_acc()
                        for (at_sb, atoff, vt_l, c0av, wav, jidx) in jobs:
                            first = (jidx == 0)
                            last = (jidx == nst - 1)
                            nc.tensor.matmul(
                                ops_[:, c0av:c0av + wav], vt_l,
                                at_sb[:, atoff:atoff + wav],
                                start=first, stop=last)
                            nc.tensor.matmul(
                                pd[:, c0av:c0av + wav], ones_sb[:, 0:1],
                                at_sb[:, atoff:atoff + wav],
                                start=first, stop=last)

                    for ui, unit in enumerate(units):
                        sc = pssc.tile([128, 1024], f32, tag="sc", name="sc")
                        at_t = atp.tile([128, 1024], f32r, tag="at",
                                        name="at")
                        jobs = []
                        off = 0
                        for j in unit:
                            src, t, col0, wsc, mspec = steps[j]
                            if src == 'kb':
                                lhsT = kbk_sb[:, 128 * t:128 * t + 128]
                                rhs = qnT[:, qcol:qcol + 512]
                                vt_l = kbv_sb[:, 128 * t:128 * t + 128]
                            else:
                                lhsT = kTr[:, 128 * t:128 * t + 128]
                                rhs = qTr[:, qcol + col0:qcol + 512]
                                vt_l = vkm[:, 128 * t:128 * t + 128]
                            nc.tensor.matmul(sc[:, off:off + wsc], lhsT, rhs,
                                             start=True, stop=True)
                            if mspec is not None and mspec[0] == 'tri':
                                _, toff, wadd = mspec
                                nc.vector.tensor_add(
                                    sc[:, off:off + wadd],
                                    sc[:, off:off + wadd],
                                    tri_sb[:, toff:toff + wadd])
                            elif mspec is not None and mspec[0] == 'packed':
                                _, kidx, wadd = mspec
                                nc.vector.tensor_add(
                                    sc[:, off:off + wadd],
                                    sc[:, off:off + wadd],
                                    mask_sb[:, 512 * kidx:512 * kidx + wadd])
                            # widened tiles: leading 128 cols are masked to
                            # exp()=0; av/den include them (adding zeros)
                            jobs.append((at_t, off, vt_l, col0, wsc, j))
                            off += wsc
                        nc.scalar.activation(at_t[:, 0:off], sc[:, 0:off],
                                             AF.Exp, scale=SCALE)
                        if ui == 0 and defA[0] is not None:
                            defA[0]()
                            defA[0] = None
                        if ui == 1:
                            if defB[0] is not None:
                                defB[0]()
                                defB[0] = None
                            if c == 1:
                                # y tile i needs only c=0 outn: emit here
                                # so its DMA overlaps this head's attention
                                emit_y_tile(i)
                        # flush with a lag of TWO exp units so the av/den
                        # matmuls never wait on the scores->mask->exp chain
                        if len(pending) >= 2:
                            flush(pending.pop(0))
                        pending.append(jobs)
                    for jb in pending:
                        flush(jb)
                    ops_, pd = get_acc()

                    def normA(pd=pd):
                        rec32 = nrm.tile([1, 512], f32, tag="rec32")
                        nc.vector.reciprocal_approx_fast(rec32[:], pd[:])
                        rec = nrm.tile([1, 512], f32r, tag="rec")
                        nc.vector.tensor_copy(rec[:], rec32[:])
                        return rec

                    recs = {}

                    def defA_fn(recs=recs, normA=normA):
                        recs['rec'] = normA()

                    def defB_fn(recs=recs, ops_=ops_, qcol=qcol, c=c, i=i):
                        rec = recs['rec']
                        # bc shares the psden bank-slot ring with pd: the
                        # slot alternates pd(h) -> bc(h) -> pd(h+1) with
                        # WAR chains recip(h) / bc_sb-copy(h) in between
                        bc = psden.tile([128, 512], f32, tag="den",
                                        name="bcp")
                        nc.tensor.matmul(bc[:], ones_sb[0:1, :],
                                         rec[:], start=True, stop=True)
                        bc_sb = nrm.tile([128, 512], f32r, tag="bc_sb")
                        nc.vector.tensor_copy(bc_sb[:], bc[:])
                        nc.vector.tensor_mul(outn[:, qcol:qcol + 512],
                                             ops_[:], bc_sb[:])
                        if c == 1 and i == 2:
                            # heads 0-2 done: emit partial y tiles 4-7
                            for st in range(4, 8):
                                emit_y_partial(st)

                    defA[0] = defA_fn
                    defB[0] = defB_fn
            defA[0]()
            defB[0]()
            for st in range(4, 8):
                emit_y_final(st)

    nc.compile()
    return nc


def kernel(hidden_states, attention_mask, position_ids, kb_keys, kb_values,
           Wq, Wq_new, Wk, Wv, Wo):
    from concourse.bass_utils import run_bass_kernel_spmd

    hidden_states = np.asarray(hidden_states, dtype=np.float32)
    attention_mask = np.asarray(attention_mask, dtype=np.float32)
    position_ids = np.asarray(position_ids)
    kb_keys = np.asarray(kb_keys, dtype=np.float32)
    kb_values = np.asarray(kb_values, dtype=np.float32)
    Wq = np.asarray(Wq, dtype=np.float32)
    Wq_new = np.asarray(Wq_new, dtype=np.float32)
    Wk = np.asarray(Wk, dtype=np.float32)
    Wv = np.asarray(Wv, dtype=np.float32)
    Wo = np.asarray(Wo, dtype=np.float32)

    # ---- host: classify self-attention mask blocks ----
    mask = attention_mask[:, 0]  # (B, S, S) [q, key]
    tri_blk = np.where(
        np.arange(128)[None, :] >= np.arange(128)[:, None], 0.0,
        NEG).astype(np.float32)  # [key, q] triangle

    self_steps = {}
    packed = []  # (b-independent) packed fallback mask blocks, [key, q]
    use_tri = False
    for c in range(2):
        lst = []
        for t in range(8):
            blk = mask[:, 512 * c:512 * c + 512, 128 * t:128 * t + 128]
            if np.all(blk <= -1e8):
                continue
            colmask = np.all(blk <= -1e8, axis=(0, 2))  # (512,) per q-col
            col0 = 0
            while col0 < 512 and colmask[col0]:
                col0 += 1
            col0 = (col0 // 128) * 128
            sub = blk[:, col0:, :]  # (B, w, 128) [q, key]
            if not np.any(sub < 0):
                lst.append((t, col0, None, False))
                continue
            # mixed: is it the canonical causal triangle at window start?
            w = 512 - col0
            exp_pat = np.zeros((w, 128), np.float32)
            exp_pat[:128] = tri_blk.T  # [q, key]
            is_tri = all(np.array_equal(sub[b_], exp_pat) for b_ in range(B))
            if is_tri:
                use_tri = True
                if w == 128:
                    # widen to 256 (f32r <256-col moving runs at 1/4 rate);
                    # extra leading block is fully masked -> exp gives 0
                    lst.append((t, col0 - 128, ('tri', 0, 256), True))
                else:
                    lst.append((t, col0, ('tri', 128, 128), False))
            else:
                packed.append((c, t, col0))
                lst.append((t, col0, ('packed', len(packed) - 1, w), False))
        self_steps[c] = lst
    n_mask = len(packed)

    plan = dict(self_steps=self_steps, n_mask=n_mask, use_tri=use_tri)
    nc = _build_program(plan)

    # ---- host: shared constant prep ----
    inv_freq = 1.0 / (THETA ** (np.arange(0, HD, 2, dtype=np.float32) / HD))
    P = np.zeros((HD, HD), np.float32)
    for d in range(64):
        P[d, d + 64] = -1.0
        P[d + 64, d] = 1.0
    ropePT = np.ascontiguousarray(P.T)
    ident = np.eye(128, dtype=np.float32)
    ones_np = np.ones((128, 128), np.float32)
    tri384 = np.concatenate([np.full((128, 128), NEG, np.float32),
                             tri_blk, np.zeros((128, 128), np.float32)],
                            axis=1)

    def pack_w(wT, ndt):
        # wT (H, 128*ndt) -> (128, 2048*ndt): tile (dt) block holds 16
        # h-tiles side by side: cols 2048*dt + 128*h = wT[128h:+128, 128dt:+128]
        out = np.empty((128, 2048 * ndt), np.float32)
        for dt_i in range(ndt):
            for h in range(16):
                out[:, 2048 * dt_i + 128 * h:2048 * dt_i + 128 * h + 128] = \
                    wT[128 * h:128 * h + 128, 128 * dt_i:128 * dt_i + 128]
        return out

    cosTs, sinTs, maskTs = [], [], []
    for b in range(B):
        freqs = position_ids[b].astype(np.float32)[:, None] * inv_freq[None, :]
        emb = np.concatenate([freqs, freqs], axis=1)  # (S, 128)
        cosTs.append(np.ascontiguousarray(np.cos(emb).T.astype(np.float32)))
        sinTs.append(np.ascontiguousarray(np.sin(emb).T.astype(np.float32)))
        if n_mask:
            mt = np.zeros((128, 512 * n_mask), np.float32)
            for kidx, (c, t, col0) in enumerate(packed):
                w = 512 - col0
                mt[:, 512 * kidx:512 * kidx + w] = \
                    mask[b, 512 * c + col0:512 * c + 512,
                         128 * t:128 * t + 128].T
            maskTs.append(mt)

    in_maps = []
    for cid in range(8):
        b, g = cid // 4, cid % 4
        kbv_p = np.empty((128, KB), np.float32)
        kvb = kb_values[b, :, 128 * g:128 * g + 128]
        for t in range(8):
            kbv_p[:, 128 * t:128 * t + 128] = kvb[128 * t:128 * t + 128, :]
        wo_p = np.empty((128, 8192), np.float32)
        woT = Wo[:, 512 * g:512 * g + 512].T  # (512, 2048)
        for i in range(4):
            wo_p[:, 2048 * i:2048 * i + 2048] = woT[128 * i:128 * i + 128, :]
        m = dict(
            xT=np.ascontiguousarray(hidden_states[b].T),
            wq=pack_w(Wq[512 * g:512 * g + 512, :].T, 4),
            wqn=pack_w(Wq_new[512 * g:512 * g + 512, :].T, 4),
            wk=pack_w(Wk[128 * g:128 * g + 128, :].T, 1),
            wv=pack_w(Wv[128 * g:128 * g + 128, :].T, 1),
            wo=wo_p,
            kbkT=np.ascontiguousarray(kb_keys[b, :, 128 * g:128 * g + 128].T),
            kbv=kbv_p,
            cosT=cosTs[b], sinT=sinTs[b],
            ropePT=ropePT, ident=ident, ones=ones_np,
        )
        if use_tri:
            m['tri'] = tri384
        if n_mask:
            m['masks'] = maskTs[b]
        in_maps.append(m)

    res = run_bass_kernel_spmd(nc, in_maps, core_ids=list(range(8)))
    global LAST_RESULTS
    LAST_RESULTS = res

    out = np.zeros((B, S, H), np.float32)
    for cid in range(8):
        b = cid // 4
        out[b] += res.results[cid]["y"]
        out[b, S // 2:] += res.results[cid]["y3"].astype(np.float32)
    return out
